# revision 4
# baseline (speedup 1.0000x reference)
"""Trainium2 Bass SPMD kernel for nn_GCM_23390391894818.

8 NeuronCores, one SPMD program; per-core behavior differs only through input
data (slices, per-core interp weights, dynamic pn-window offsets).

Sharding: core k owns 288 tokens (= 6 rows of the 48x48 pooled grid) for all
channel-major linears and the attention (transposed, unnormalized-exp form),
and 12 rows of the 96x96 grid for the upsample+dsc stage.  Collectives:
AllGather Pl/Ph/V (bf16), dyn, pn (fp32); AllReduce for the attention
column-mean.  All matmuls bf16 with fp32 PSUM accumulation; BN folded into
pointwise weights on host; residual/LN/upsample in fp32.
"""

import sys

for _p in ("/opt/trn_rl_repo",):
    if _p not in sys.path:
        sys.path.insert(0, _p)

import numpy as np
import ml_dtypes

import concourse.bass as bass
import concourse.mybir as mybir
import concourse.tile as tile
from concourse import bacc
from concourse.bass_utils import run_bass_kernel_spmd
from concourse.tile_rust import add_dep_helper
from concourse.masks import make_identity

P = 128
C = 2304
CT = C // P            # 18 channel tiles
NCORE = 8
COLS = 288             # own tokens / columns per core (48-res)
WIN48 = 10             # pooled window rows: [6k-2, 6k+8)
WINC = WIN48 * 48      # 480
PNW = 10               # pn window rows48: [6k-2, 6k+8)
UPR = 14               # up window rows96: [12k-1, 12k+13)
OUTR = 12
OUTC = OUTR * 96       # 1152
JC = [(0, 128), (128, 128), (256, 32)]
NCH5 = [(0, 512), (512, 512), (1024, 512), (1536, 512), (2048, 256)]
NCH3 = [(0, 512), (512, 512), (1024, 128)]
SCALE = C ** -0.5
EPS = 1e-5

f32 = mybir.dt.float32
bf16 = mybir.dt.bfloat16
u32 = mybir.dt.uint32
ALU = mybir.AluOpType
ACT = mybir.ActivationFunctionType

BF = np.dtype(ml_dtypes.bfloat16)


def _ap(h, offset, pattern):
    return bass.AP(tensor=h, offset=offset, ap=[list(x) for x in pattern])


def _dyn_ap(h, offset_scalar, pattern):
    return bass.AP(tensor=h, offset=offset_scalar, ap=[list(x) for x in pattern],
                   dep_tracking_offset=0)


def build_program():
    nc = bacc.Bacc(target_bir_lowering=False, trn_type="TRN2")

    # ---------------- external tensors ----------------
    xp_h = nc.dram_tensor("xp", [C, WINC], bf16, kind="ExternalInput")
    xg_h = nc.dram_tensor("xg", [C, COLS], bf16, kind="ExternalInput")
    w_h = {}
    for nm in ("wq", "wk", "wl", "wsh", "wsv", "wch", "wo", "wd"):
        w_h[nm] = nc.dram_tensor(nm, [C, C], bf16, kind="ExternalInput")
    wp_h = nc.dram_tensor("wp", [C, 384], bf16, kind="ExternalInput")
    b_h = {}
    for nm in ("bq", "bk", "bl", "bsh", "bsv", "bch", "bd"):
        b_h[nm] = nc.dram_tensor(nm, [C], f32, kind="ExternalInput")
    bp_h = nc.dram_tensor("bp", [384], f32, kind="ExternalInput")
    obrow_h = nc.dram_tensor("obrow", [P, C], f32, kind="ExternalInput")
    lng_h = nc.dram_tensor("lng", [P, C], f32, kind="ExternalInput")
    lnb_h = nc.dram_tensor("lnb", [P, C], f32, kind="ExternalInput")
    dw5h_h = nc.dram_tensor("dw5h", [C, 5], f32, kind="ExternalInput")
    dw5v_h = nc.dram_tensor("dw5v", [C, 5], f32, kind="ExternalInput")
    dw9_h = nc.dram_tensor("dw9", [C, 9], f32, kind="ExternalInput")
    dwbh_h = nc.dram_tensor("dwbh", [C], f32, kind="ExternalInput")
    dwbv_h = nc.dram_tensor("dwbv", [C], f32, kind="ExternalInput")
    dwbd_h = nc.dram_tensor("dwbd", [C], f32, kind="ExternalInput")
    whx_h = nc.dram_tensor("whx", [2 * 4 * 7 * 48], f32, kind="ExternalInput")
    wwx_h = nc.dram_tensor("wwx", [2 * 3 * 14 * 48], f32, kind="ExternalInput")
    meta_h = nc.dram_tensor("meta", [1, 8], u32, kind="ExternalInput")

    out_h = nc.dram_tensor("out", [C, OUTC], f32, kind="ExternalOutput")

    # ---------------- internal DRAM ----------------
    agpl_i = nc.dram_tensor("agpl_i", [C, COLS], bf16)
    agpl_o = nc.dram_tensor("agpl_o", [NCORE * C, COLS], bf16, addr_space="Shared")
    agph_i = nc.dram_tensor("agph_i", [C, COLS], bf16)
    agph_o = nc.dram_tensor("agph_o", [NCORE * C, COLS], bf16, addr_space="Shared")
    agv_i = nc.dram_tensor("agv_i", [COLS, C], bf16)
    agv_o = nc.dram_tensor("agv_o", [NCORE * COLS, C], bf16, addr_space="Shared")
    ard_i = nc.dram_tensor("ard_i", [C], f32)
    ard_o = nc.dram_tensor("ard_o", [C], f32, addr_space="Shared")
    agdy_i = nc.dram_tensor("agdy_i", [COLS], f32)
    agdy_o = nc.dram_tensor("agdy_o", [NCORE * COLS], f32, addr_space="Shared")
    agpn_i = nc.dram_tensor("agpn_i", [C, COLS], f32)
    agpn_o = nc.dram_tensor("agpn_o", [NCORE * C, COLS], f32, addr_space="Shared")
    zd_d = nc.dram_tensor("zd", [2, 384], f32)
    vown_d = nc.dram_tensor("vown", [COLS, C], f32)
    upb_d = nc.dram_tensor("upb", [C, OUTC], f32)

    RG = [list(range(NCORE))]

    sp = nc.engines[mybir.EngineType.SP]
    moff = []
    for i in range(3):
        r = sp.alloc_register(f"meta_{i}")
        sp.reg_load(r, meta_h[0:1, i:i + 1])
        moff.append(nc.snap(r, donate=True, min_val=0,
                            max_val=NCORE * C * COLS))

    with tile.TileContext(nc) as tc:
      with tc.tile_pool(name="consts", bufs=1) as cp:
        bt = {}
        for nm in ("bq", "bk", "bl", "bsh", "bsv", "bch", "bd"):
            t = cp.tile([P, CT], f32, tag=f"c_{nm}")
            nc.sync.dma_start(t[:], _ap(b_h[nm], 0, [(1, P), (P, CT)]))
            bt[nm] = t
        bp_t = cp.tile([P, 3], f32, tag="c_bp")
        nc.sync.dma_start(bp_t[:], _ap(bp_h, 0, [(1, P), (P, 3)]))
        dw9_t = cp.tile([P, CT, 9], f32, tag="c_dw9")
        nc.sync.dma_start(dw9_t[:], _ap(dw9_h, 0, [(9, P), (9 * P, CT), (1, 9)]))
        dwbd_t = cp.tile([P, CT], f32, tag="c_dwbd")
        nc.sync.dma_start(dwbd_t[:], _ap(dwbd_h, 0, [(1, P), (P, CT)]))
        ones_t = cp.tile([P, 1], bf16, tag="c_ones")
        nc.vector.memset(ones_t[:], 1.0)
        ident_t = cp.tile([P, P], f32, tag="c_ident")
        make_identity(nc, ident_t[:])

        with tc.tile_pool(name="poolB", bufs=1) as pb:
          plph_t = pb.tile([P, CT, COLS], f32, tag="plph")
          u_t = pb.tile([P, CT, COLS], bf16, tag="u")
          with tc.tile_pool(name="poolB2", bufs=1) as pb2:
            plbf_t = pb2.tile([P, CT, COLS], bf16, tag="plbf")
            phbf_t = pb2.tile([P, CT, COLS], bf16, tag="phbf")

            with tc.tile_pool(name="poolA", bufs=1) as pa:
              xp_t = pa.tile([P, CT, WINC], bf16, tag="xp")
              nc.sync.dma_start(
                  xp_t[:], _ap(xp_h, 0, [(WINC, P), (WINC * P, CT), (1, WINC)]))
              xg_t = pa.tile([P, CT, COLS], bf16, tag="xg")
              nc.sync.dma_start(
                  xg_t[:], _ap(xg_h, 0, [(COLS, P), (COLS * P, CT), (1, COLS)]))

              # ===== phase V: V_token (own rows, token-major) =====
              vw_dmas = []
              with tc.tile_pool(name="poolV", bufs=1) as pv, \
                   tc.tile_pool(name="poolVs", bufs=3) as pvs, \
                   tc.tile_pool(name="psV", bufs=4, space="PSUM") as psv:
                vf_t = pv.tile([P, 3, C], f32, tag="vf")
                vbf_t = pv.tile([P, 3, C], bf16, tag="vbf")
                obrow_t = pv.tile([P, C], f32, tag="obrow")
                nc.sync.dma_start(obrow_t[:], obrow_h.ap())
                for n0, nw in NCH5:
                    pss = [psv.tile([P, 512], f32, tag="ps", name=f"psv_{n0}_{_j}") for _j in range(3)]
                    for k in range(CT):
                        wt = pvs.tile([P, 512], bf16, tag="vw")
                        nc.sync.dma_start(
                            wt[:, :nw],
                            _ap(w_h["wo"], k * P * C + n0, [(C, P), (1, nw)]))
                        for j, (j0, nj) in enumerate(JC):
                            nc.tensor.matmul(
                                pss[j][:nj, :nw],
                                xp_t[:, k, 96 + j0:96 + j0 + nj],
                                wt[:, :nw], start=(k == 0), stop=(k == CT - 1))
                    for j, (j0, nj) in enumerate(JC):
                        nc.vector.tensor_add(vf_t[:nj, j, n0:n0 + nw],
                                             pss[j][:nj, :nw],
                                             obrow_t[:nj, n0:n0 + nw])
                        nc.vector.tensor_copy(vbf_t[:nj, j, n0:n0 + nw],
                                              vf_t[:nj, j, n0:n0 + nw])
                dvs = []
                for j, (j0, nj) in enumerate(JC):
                    dvs.append(nc.sync.dma_start(
                        _ap(agv_i, j0 * C, [(C, nj), (1, C)]), vbf_t[:nj, j, :]))
                    vw_dmas.append(nc.sync.dma_start(
                        _ap(vown_d, j0 * C, [(C, nj), (1, C)]), vf_t[:nj, j, :]))
                c_v = nc.gpsimd.collective_compute(
                    "AllGather", ALU.bypass, replica_groups=RG,
                    ins=[agv_i.ap().opt()], outs=[agv_o.ap().opt()])
                for dv in dvs:
                    add_dep_helper(c_v.ins, dv.ins)

              # ===== phase 1+2: depthwise 5-taps, gelu, six linears =====
              with tc.tile_pool(name="poolL", bufs=1) as pl, \
                   tc.tile_pool(name="poolLs", bufs=3) as pls, \
                   tc.tile_pool(name="psL", bufs=4, space="PSUM") as psl:
                gh_t = pl.tile([P, CT, COLS], bf16, tag="gh")
                gv_t = pl.tile([P, CT, COLS], bf16, tag="gv")
                dw5h_t = pl.tile([P, CT, 5], f32, tag="dw5h")
                nc.sync.dma_start(
                    dw5h_t[:], _ap(dw5h_h, 0, [(5, P), (5 * P, CT), (1, 5)]))
                dw5v_t = pl.tile([P, CT, 5], f32, tag="dw5v")
                nc.sync.dma_start(
                    dw5v_t[:], _ap(dw5v_h, 0, [(5, P), (5 * P, CT), (1, 5)]))
                dwbh_t = pl.tile([P, CT], f32, tag="dwbh")
                nc.sync.dma_start(dwbh_t[:], _ap(dwbh_h, 0, [(1, P), (P, CT)]))
                dwbv_t = pl.tile([P, CT], f32, tag="dwbv")
                nc.sync.dma_start(dwbv_t[:], _ap(dwbv_h, 0, [(1, P), (P, CT)]))

                for t in range(CT):
                    xpw = xp_t[:, t, :].rearrange("p (r w) -> p r w", w=48)
                    acc = pls.tile([P, 6, 48], bf16, tag="dwacc")
                    tmp = pls.tile([P, 6, 48], bf16, tag="dwtmp")
                    for tap in range(5):
                        src = xpw[:, tap:tap + 6, :]
                        if tap == 0:
                            nc.vector.tensor_scalar(
                                acc[:], src, dw5h_t[:, t, 0:1], None, ALU.mult)
                        else:
                            nc.vector.tensor_scalar(
                                tmp[:], src, dw5h_t[:, t, tap:tap + 1], None,
                                ALU.mult)
                            nc.vector.tensor_add(acc[:], acc[:], tmp[:])
                    nc.scalar.activation(
                        gh_t[:, t, :].rearrange("p (r w) -> p r w", w=48), acc[:],
                        ACT.Gelu, bias=dwbh_t[:, t:t + 1])
                    acc2 = pls.tile([P, 6, 48], bf16, tag="dwacc2")
                    nc.vector.memset(acc2[:], 0.0)
                    for tap in range(5):
                        s = tap - 2
                        j0 = max(0, -s)
                        w = 48 - abs(s)
                        src = xpw[:, 2:8, j0 + s:j0 + s + w]
                        tm2 = tmp[:, :, j0:j0 + w]
                        nc.vector.tensor_scalar(
                            tm2, src, dw5v_t[:, t, tap:tap + 1], None, ALU.mult)
                        nc.vector.tensor_add(acc2[:, :, j0:j0 + w],
                                             acc2[:, :, j0:j0 + w], tm2)
                    nc.scalar.activation(
                        gv_t[:, t, :].rearrange("p (r w) -> p r w", w=48), acc2[:],
                        ACT.Gelu, bias=dwbv_t[:, t:t + 1])

                sig_t = pl.tile([P, CT, COLS], bf16, tag="sig")
                xv_t = pl.tile([P, CT, COLS], bf16, tag="xv")
                sum_t = pl.tile([P, CT, COLS], bf16, tag="sumhv")

                def colblock(wh, rhs_t, epilogue):
                    for m in range(CT):
                        wt = pls.tile([P, CT, P], bf16, tag="wstream")
                        nc.sync.dma_start(
                            wt[:], _ap(wh, m * P, [(C, P), (C * P, CT), (1, P)]))
                        ps = psl.tile([P, 512], f32, tag="ps")
                        for k in range(CT):
                            nc.tensor.matmul(ps[:, :COLS], wt[:, k, :],
                                             rhs_t[:, k, :], start=(k == 0),
                                             stop=(k == CT - 1))
                        epilogue(m, ps[:, :COLS])

                def ep_q(m, ps):
                    nc.scalar.activation(sig_t[:, m, :], ps, ACT.Sigmoid,
                                         bias=bt["bq"][:, m:m + 1])
                colblock(w_h["wq"], xg_t, ep_q)

                def ep_k(m, ps):
                    tm = pls.tile([P, COLS], bf16, tag="ep_tmp")
                    nc.scalar.activation(tm[:], ps, ACT.Identity,
                                         bias=bt["bk"][:, m:m + 1])
                    nc.vector.tensor_mul(tm[:], tm[:], sig_t[:, m, :])
                    nc.vector.tensor_add(xv_t[:, m, :], tm[:], xg_t[:, m, :])
                colblock(w_h["wk"], xg_t, ep_k)

                def ep_l(m, ps):
                    nc.scalar.activation(plph_t[:, m, :], ps, ACT.Identity,
                                         bias=bt["bl"][:, m:m + 1])
                    nc.vector.tensor_copy(plbf_t[:, m, :], plph_t[:, m, :])
                colblock(w_h["wl"], xv_t, ep_l)

                def ep_sh(m, ps):
                    nc.scalar.activation(sum_t[:, m, :], ps, ACT.Identity,
                                         bias=bt["bsh"][:, m:m + 1])
                colblock(w_h["wsh"], gh_t, ep_sh)

                def ep_sv(m, ps):
                    tm = pls.tile([P, COLS], bf16, tag="ep_tmp")
                    nc.scalar.activation(tm[:], ps, ACT.Identity,
                                         bias=bt["bsv"][:, m:m + 1])
                    nc.vector.tensor_add(sum_t[:, m, :], sum_t[:, m, :], tm[:])
                colblock(w_h["wsv"], gv_t, ep_sv)

                def ep_ch(m, ps):
                    tm = pls.tile([P, COLS], f32, tag="ep_tmp32")
                    nc.scalar.activation(tm[:], ps, ACT.Identity,
                                         bias=bt["bch"][:, m:m + 1])
                    nc.vector.tensor_copy(phbf_t[:, m, :], tm[:])
                    nc.vector.tensor_add(plph_t[:, m, :], plph_t[:, m, :], tm[:])
                colblock(w_h["wch"], sum_t, ep_ch)

                d1 = nc.sync.dma_start(
                    _ap(agpl_i, 0, [(COLS, P), (COLS * P, CT), (1, COLS)]),
                    plbf_t[:])
                d2 = nc.sync.dma_start(
                    _ap(agph_i, 0, [(COLS, P), (COLS * P, CT), (1, COLS)]),
                    phbf_t[:])
                c_pl = nc.gpsimd.collective_compute(
                    "AllGather", ALU.bypass, replica_groups=RG,
                    ins=[agpl_i.ap().opt()], outs=[agpl_o.ap().opt()])
                c_ph = nc.gpsimd.collective_compute(
                    "AllGather", ALU.bypass, replica_groups=RG,
                    ins=[agph_i.ap().opt()], outs=[agph_o.ap().opt()])
                add_dep_helper(c_pl.ins, d1.ins)
                add_dep_helper(c_ph.ins, d2.ins)
            # poolA closed (xp/xg/linear temps freed)

            # ===== phase 3: attention =====
            with tc.tile_pool(name="poolC", bufs=1) as pc, \
                 tc.tile_pool(name="poolCs", bufs=3) as pcs, \
                 tc.tile_pool(name="psC", bufs=4, space="PSUM") as psc, \
                 tc.tile_pool(name="psCs", bufs=2, space="PSUM") as pscs:
                e1_t = pc.tile([P, CT, COLS], bf16, tag="e1")
                e2_t = pc.tile([P, CT, COLS], bf16, tag="e2")

                def eblock(ago, coll, rhs_t, eout):
                    for m in range(CT):
                        wt = pcs.tile([P, CT, P], bf16, tag="estream")
                        c0 = m * P
                        while c0 < (m + 1) * P:
                            r = c0 // COLS
                            ce = min((m + 1) * P, (r + 1) * COLS)
                            dm = nc.sync.dma_start(
                                wt[:, :, c0 - m * P:ce - m * P],
                                _ap(ago, (r * C) * COLS + (c0 - r * COLS),
                                    [(COLS, P), (P * COLS, CT), (1, ce - c0)]))
                            add_dep_helper(dm.ins, coll.ins)
                            c0 = ce
                        ps = psc.tile([P, 512], f32, tag="ps")
                        for k in range(CT):
                            nc.tensor.matmul(ps[:, :COLS], wt[:, k, :],
                                             rhs_t[:, k, :], start=(k == 0),
                                             stop=(k == CT - 1))
                        nc.scalar.activation(eout[:, m, :], ps[:, :COLS],
                                             ACT.Exp, scale=float(SCALE))

                eblock(agpl_o, c_pl, phbf_t, e1_t)
                eblock(agph_o, c_ph, plbf_t, e2_t)

                zc_t = pc.tile([P, 2, 3], f32, tag="zc")
                for ei, et in ((0, e1_t), (1, e2_t)):
                    for j, (j0, nj) in enumerate(JC):
                        psz = pscs.tile([P, P], f32, tag="small")
                        for k in range(CT):
                            nc.tensor.matmul(psz[:nj, 0:1], et[:, k, j0:j0 + nj],
                                             ones_t[:], start=(k == 0),
                                             stop=(k == CT - 1))
                        nc.vector.tensor_copy(zc_t[:nj, ei, j:j + 1],
                                              psz[:nj, 0:1])
                nc.vector.reciprocal(zc_t[:], zc_t[:])
                dzw = nc.sync.dma_start(
                    _ap(zd_d, 0, [(1, P), (384, 2), (P, 3)]), zc_t[:])
                zr_t = pc.tile([P, 2, COLS], f32, tag="zrow")
                dzr = nc.sync.dma_start(
                    zr_t[:], _ap(zd_d, 0, [(0, P), (384, 2), (1, COLS)]))
                add_dep_helper(dzr.ins, dzw.ins)
                zb_t = pc.tile([P, 2, COLS], bf16, tag="zrowb")
                nc.vector.tensor_copy(zb_t[:], zr_t[:])

                dsum_t = pc.tile([P, CT], f32, tag="dsum")
                for k in range(CT):
                    tmu = pcs.tile([P, COLS], bf16, tag="utmp")
                    nc.vector.tensor_mul(u_t[:, k, :], e1_t[:, k, :],
                                         zb_t[:, 0, :])
                    nc.vector.tensor_mul(tmu[:], e2_t[:, k, :], zb_t[:, 1, :])
                    nc.vector.tensor_add(u_t[:, k, :], u_t[:, k, :], tmu[:])
                    nc.vector.tensor_reduce(dsum_t[:, k:k + 1], u_t[:, k, :],
                                            axis=mybir.AxisListType.X,
                                            op=ALU.add)
                ddw = nc.sync.dma_start(_ap(ard_i, 0, [(1, P), (P, CT)]),
                                        dsum_t[:])
                c_d = nc.gpsimd.collective_compute(
                    "AllReduce", ALU.add, replica_groups=RG,
                    ins=[ard_i.ap().opt()], outs=[ard_o.ap().opt()])
                add_dep_helper(c_d.ins, ddw.ins)

                dT_t = pc.tile([P, CT], f32, tag="dT")
                drd = nc.sync.dma_start(dT_t[:], _ap(ard_o, 0, [(1, P), (P, CT)]))
                add_dep_helper(drd.ins, c_d.ins)
                dbf_t = pc.tile([P, CT], bf16, tag="dbf")
                nc.vector.tensor_scalar(dbf_t[:], dT_t[:], 1.0 / C, None,
                                        ALU.mult)
                wpt = pc.tile([P, CT, 384], bf16, tag="wpt")
                nc.sync.dma_start(
                    wpt[:], _ap(wp_h, 0, [(384, P), (384 * P, CT), (1, 384)]))
                dyv_t = pc.tile([P, 3], f32, tag="dyv")
                for j, (j0, nj) in enumerate(JC):
                    psd = pscs.tile([P, P], f32, tag="small")
                    for k in range(CT):
                        nc.tensor.matmul(psd[:, 0:1], wpt[:, k, j0:j0 + P],
                                         dbf_t[:, k:k + 1], start=(k == 0),
                                         stop=(k == CT - 1))
                    nc.vector.tensor_copy(dyv_t[:, j:j + 1], psd[:, 0:1])
                nc.vector.tensor_add(dyv_t[:], dyv_t[:], bp_t[:])
                dyws = []
                for j, (j0, nj) in enumerate(JC):
                    dyws.append(nc.sync.dma_start(
                        _ap(agdy_i, j0, [(1, nj)]), dyv_t[:nj, j:j + 1]))
                c_dy = nc.gpsimd.collective_compute(
                    "AllGather", ALU.bypass, replica_groups=RG,
                    ins=[agdy_i.ap().opt()], outs=[agdy_o.ap().opt()])
                for dyw in dyws:
                    add_dep_helper(c_dy.ins, dyw.ins)

            # ===== phase 4: numer + prompt + LN + pn =====
            with tc.tile_pool(name="poolD", bufs=1) as pd, \
                 tc.tile_pool(name="poolDs", bufs=3) as pds, \
                 tc.tile_pool(name="psD", bufs=4, space="PSUM") as psd4, \
                 tc.tile_pool(name="psDs", bufs=2, space="PSUM") as psds:
                pf_t = pd.tile([P, 3, C], f32, tag="pf")
                pno_t = pd.tile([P, CT, COLS], f32, tag="pno")
                dynb_t = pd.tile([P, C], f32, tag="dynb")
                drb = nc.sync.dma_start(dynb_t[:],
                                        _ap(agdy_o, 0, [(0, P), (1, C)]))
                add_dep_helper(drb.ins, c_dy.ins)
                lng_t = pd.tile([P, C], f32, tag="lng")
                nc.sync.dma_start(lng_t[:], lng_h.ap())
                lnb_t = pd.tile([P, C], f32, tag="lnb")
                nc.sync.dma_start(lnb_t[:], lnb_h.ap())

                for n0, nw in NCH5:
                    pss = [psd4.tile([P, 512], f32, tag="ps", name=f"psd_{n0}_{_j}") for _j in range(3)]
                    for k in range(CT):
                        vt = pds.tile([P, 512], bf16, tag="vstream")
                        r0 = k * P
                        while r0 < (k + 1) * P:
                            r = r0 // COLS
                            re = min((k + 1) * P, (r + 1) * COLS)
                            dm = nc.sync.dma_start(
                                vt[r0 - k * P:re - k * P, :nw],
                                _ap(agv_o, r0 * C + n0, [(C, re - r0), (1, nw)]))
                            add_dep_helper(dm.ins, c_v.ins)
                            r0 = re
                        for j, (j0, nj) in enumerate(JC):
                            nc.tensor.matmul(pss[j][:nj, :nw],
                                             u_t[:, k, j0:j0 + nj], vt[:, :nw],
                                             start=(k == 0), stop=(k == CT - 1))
                    for j, (j0, nj) in enumerate(JC):
                        vr = pds.tile([P, 512], f32, tag="vread")
                        dv = nc.sync.dma_start(
                            vr[:nj, :nw],
                            _ap(vown_d, j0 * C + n0, [(C, nj), (1, nw)]))
                        for wdma in vw_dmas:
                            add_dep_helper(dv.ins, wdma.ins)
                        nc.vector.tensor_mul(pf_t[:nj, j, n0:n0 + nw],
                                             pss[j][:nj, :nw],
                                             dynb_t[:nj, n0:n0 + nw])
                        nc.vector.tensor_add(pf_t[:nj, j, n0:n0 + nw],
                                             pf_t[:nj, j, n0:n0 + nw],
                                             vr[:nj, :nw])

                sq_t = pd.tile([P, C], f32, tag="sq")
                for j, (j0, nj) in enumerate(JC):
                    s1 = pds.tile([P, 1], f32, tag="s1")
                    nc.vector.tensor_reduce(s1[:nj], pf_t[:nj, j, :],
                                            axis=mybir.AxisListType.X,
                                            op=ALU.add)
                    nc.vector.tensor_scalar(s1[:nj], s1[:nj], -1.0 / C, None,
                                            ALU.mult)
                    nc.vector.tensor_scalar(pf_t[:nj, j, :], pf_t[:nj, j, :],
                                            s1[:nj], None, ALU.add)
                    nc.scalar.activation(sq_t[:nj], pf_t[:nj, j, :], ACT.Square)
                    v1 = pds.tile([P, 1], f32, tag="v1")
                    nc.vector.tensor_reduce(v1[:nj], sq_t[:nj],
                                            axis=mybir.AxisListType.X,
                                            op=ALU.add)
                    nc.vector.tensor_scalar(v1[:nj], v1[:nj], 1.0 / C, EPS,
                                            ALU.mult, ALU.add)
                    nc.scalar.activation(v1[:nj], v1[:nj], ACT.Sqrt)
                    nc.vector.reciprocal(v1[:nj], v1[:nj])
                    nc.vector.tensor_scalar(pf_t[:nj, j, :], pf_t[:nj, j, :],
                                            v1[:nj], None, ALU.mult)
                    nc.vector.tensor_mul(pf_t[:nj, j, :], pf_t[:nj, j, :],
                                         lng_t[:nj, :])
                    nc.vector.tensor_add(pf_t[:nj, j, :], pf_t[:nj, j, :],
                                         lnb_t[:nj, :])

                for j, (j0, nj) in enumerate(JC):
                    for t in range(CT):
                        pst = psds.tile([P, P], f32, tag="small")
                        nc.tensor.matmul(pst[:, :nj],
                                         pf_t[:nj, j, t * P:(t + 1) * P],
                                         ident_t[:nj, :nj], is_transpose=True,
                                         start=True, stop=True)
                        nc.vector.tensor_add(pno_t[:, t, j0:j0 + nj],
                                             pst[:, :nj],
                                             plph_t[:, t, j0:j0 + nj])
                dpn = nc.sync.dma_start(
                    _ap(agpn_i, 0, [(COLS, P), (COLS * P, CT), (1, COLS)]),
                    pno_t[:])
                c_pn = nc.gpsimd.collective_compute(
                    "AllGather", ALU.bypass, replica_groups=RG,
                    ins=[agpn_i.ap().opt()], outs=[agpn_o.ap().opt()])
                add_dep_helper(c_pn.ins, dpn.ins)

        # poolB/B2 closed
        # ===== phase 5: upsample + dsc =====
        with tc.tile_pool(name="poolE", bufs=1) as pe, \
             tc.tile_pool(name="poolEs", bufs=2) as pes, \
             tc.tile_pool(name="poolEw", bufs=3) as pew, \
             tc.tile_pool(name="psE", bufs=4, space="PSUM") as pse:
            g_t = pe.tile([P, CT, OUTC], bf16, tag="g")
            whx_t = pe.tile([P, 2688], f32, tag="whx")
            nc.sync.dma_start(whx_t[:], _ap(whx_h, 0, [(0, P), (1, 2688)]))
            wwx_t = pe.tile([P, 4032], f32, tag="wwx")
            nc.sync.dma_start(wwx_t[:], _ap(wwx_h, 0, [(0, P), (1, 4032)]))
            pnw_t = pe.tile([P, CT, PNW, 48], f32, tag="pnwin")
            pieces = [(0, 2), (2, 6), (8, 2)]
            for (i0, ln), off in zip(pieces, moff):
                dm = nc.sync.dma_start(
                    pnw_t[:, :, i0:i0 + ln, :],
                    _dyn_ap(agpn_o, off,
                            [(COLS, P), (P * COLS, CT), (48, ln), (1, 48)]))
                add_dep_helper(dm.ins, c_pn.ins)

            whx4 = whx_t[:].rearrange("p (g r w) -> p g r w", g=8, w=48)
            wwx4 = wwx_t[:].rearrange("p (g r w) -> p g r w", g=6, w=48)
            upw_dmas = []
            for t in range(CT):
                t96 = pes.tile([P, UPR, 50], f32, tag="t96")
                nc.vector.memset(t96[:, :, 0:1], 0.0)
                nc.vector.memset(t96[:, :, 49:50], 0.0)
                t96i = t96[:].rearrange("p (r two) c -> p two r c", two=2)
                tmh = pes.tile([P, 7, 48], f32, tag="tmh")
                for q2 in range(2):
                    dst = t96i[:, q2, :, 1:49]
                    for d in range(4):
                        src = pnw_t[:, t, d:d + 7, :]
                        wsl = whx4[:, q2 * 4 + d, :, :]
                        if d == 0:
                            nc.vector.tensor_mul(dst, src, wsl)
                        else:
                            nc.vector.tensor_mul(tmh[:], src, wsl)
                            nc.vector.tensor_add(dst, dst, tmh[:])
                up = pes.tile([P, UPR, 96], f32, tag="up")
                upi = up[:].rearrange("p r (c two) -> p two r c", two=2)
                tmw = pes.tile([P, UPR, 48], f32, tag="tmw")
                for q2 in range(2):
                    dst = upi[:, q2, :, :]
                    for d in range(3):
                        src = t96[:, :, d:d + 48]
                        wsl = wwx4[:, q2 * 3 + d, :, :]
                        if d == 0:
                            nc.vector.tensor_mul(dst, src, wsl)
                        else:
                            nc.vector.tensor_mul(tmw[:], src, wsl)
                            nc.vector.tensor_add(dst, dst, tmw[:])
                upw = nc.sync.dma_start(
                    _ap(upb_d, t * P * OUTC, [(OUTC, P), (1, OUTC)]),
                    up[:, 1:13, :])
                upw_dmas.append(upw)
                upb16 = pes.tile([P, UPR, 96], bf16, tag="upb16")
                nc.vector.tensor_copy(upb16[:], up[:])
                acc = pes.tile([P, OUTR, 96], bf16, tag="dacc")
                tmd = pes.tile([P, OUTR, 96], bf16, tag="dtmp")
                nc.vector.memset(acc[:], 0.0)
                for dy in (-1, 0, 1):
                    for dx in (-1, 0, 1):
                        tap = (dy + 1) * 3 + (dx + 1)
                        c0 = max(0, -dx)
                        w = 96 - abs(dx)
                        src = upb16[:, 1 + dy:13 + dy, c0 + dx:c0 + dx + w]
                        tms = tmd[:, :, c0:c0 + w]
                        nc.vector.tensor_scalar(
                            tms, src, dw9_t[:, t, tap:tap + 1], None, ALU.mult)
                        nc.vector.tensor_add(acc[:, :, c0:c0 + w],
                                             acc[:, :, c0:c0 + w], tms)
                nc.scalar.activation(
                    g_t[:, t, :].rearrange("p (r w) -> p r w", w=96), acc[:],
                    ACT.Gelu, bias=dwbd_t[:, t:t + 1])

            for m in range(CT):
                wt = pew.tile([P, CT, P], bf16, tag="wstream5")
                nc.sync.dma_start(
                    wt[:], _ap(w_h["wd"], m * P, [(C, P), (C * P, CT), (1, P)]))
                upo = pew.tile([P, OUTC], f32, tag="upo")
                du = nc.sync.dma_start(
                    upo[:], _ap(upb_d, m * P * OUTC, [(OUTC, P), (1, OUTC)]))
                add_dep_helper(du.ins, upw_dmas[m].ins)
                for n0, nw in NCH3:
                    ps = pse.tile([P, 512], f32, tag="ps5")
                    for k in range(CT):
                        nc.tensor.matmul(ps[:, :nw], wt[:, k, :],
                                         g_t[:, k, n0:n0 + nw],
                                         start=(k == 0), stop=(k == CT - 1))
                    of = pew.tile([P, 512], f32, tag="of")
                    nc.vector.tensor_scalar(of[:, :nw], ps[:, :nw],
                                            bt["bd"][:, m:m + 1], None, ALU.add)
                    nc.vector.tensor_add(of[:, :nw], of[:, :nw],
                                         upo[:, n0:n0 + nw])
                    nc.sync.dma_start(
                        _ap(out_h, m * P * OUTC + n0, [(OUTC, P), (1, nw)]),
                        of[:, :nw])

    nc.finalize()
    return nc


_prog_cache = {}


def _get_program():
    if "nc" not in _prog_cache:
        _prog_cache["nc"] = build_program()
    return _prog_cache["nc"]


def _rbf(a):
    return np.ascontiguousarray(a).astype(BF)


def host_inputs(x, params):
    p = {k: np.asarray(v, dtype=np.float32) for k, v in params.items()}
    x = np.asarray(x, dtype=np.float32).reshape(C, 96, 96)

    pooled = x.reshape(C, 48, 2, 48, 2).mean(axis=(2, 4))
    m = pooled.mean(axis=(1, 2))

    def bnfold(pfx):
        s = p[pfx + "_bn_g"] / np.sqrt(p[pfx + "_bn_v"] + 1e-5)
        W = p[pfx + "_pw_w"] * s[:, None]
        b = p[pfx + "_pw_b"] * s + p[pfx + "_bn_b"] - p[pfx + "_bn_m"] * s
        return W, b

    Wsh, bsh = bnfold("sch")
    Wsv, bsv = bnfold("scv")
    Wd, bd = bnfold("dsc")

    common = {
        "wq": _rbf(p["q_w"].T), "wk": _rbf(p["k_w"].T),
        "wl": _rbf(p["lin_l_w"].T), "wsh": _rbf(Wsh.T), "wsv": _rbf(Wsv.T),
        "wch": _rbf(p["convh_w"].T), "wo": _rbf(p["lin_o_w"].T),
        "wd": _rbf(Wd.T),
        "bq": p["q_b"], "bk": p["k_b"], "bl": p["lin_l_b"],
        "bsh": bsh, "bsv": bsv, "bch": p["convh_b"], "bd": bd,
        "obrow": np.ascontiguousarray(np.broadcast_to(p["lin_o_b"], (P, C))),
        "lng": np.ascontiguousarray(np.broadcast_to(p["ln_g"], (P, C))),
        "lnb": np.ascontiguousarray(np.broadcast_to(p["ln_b"], (P, C))),
        "dw5h": np.ascontiguousarray(p["sch_dw_w"][:, 0, :, 0]),
        "dw5v": np.ascontiguousarray(p["scv_dw_w"][:, 0, 0, :]),
        "dw9": np.ascontiguousarray(p["dsc_dw_w"][:, 0].reshape(C, 9)),
        "dwbh": p["sch_dw_b"], "dwbv": p["scv_dw_b"], "dwbd": p["dsc_dw_b"],
    }

    wwx = np.zeros((2, 3, 48), np.float32)
    for c in range(96):
        s = c * 47.0 / 95.0
        x0 = int(np.floor(s))
        wx = s - x0
        x1 = min(x0 + 1, 47)
        q, cc = c % 2, c // 2
        for xi, wv in ((x0, 1.0 - wx), (x1, wx)):
            d = xi + 1 - cc
            assert 0 <= d <= 2, (c, xi, cc)
            wwx[q, d, cc] += np.float32(wv)
    common["wwx"] = np.ascontiguousarray(
        np.repeat(wwx[:, :, None, :], UPR, axis=2).reshape(-1))

    in_maps = []
    for k in range(NCORE):
        d = dict(common)
        xpw = np.zeros((C, WIN48, 48), np.float32)
        lo, hi = 6 * k - 2, 6 * k + 8
        vlo, vhi = max(0, lo), min(48, hi)
        xpw[:, vlo - lo:vhi - lo, :] = pooled[:, vlo:vhi, :]
        d["xp"] = _rbf(xpw.reshape(C, WINC))
        d["xg"] = _rbf((m[:, None, None] * pooled[:, 6 * k:6 * k + 6, :])
                       .reshape(C, COLS))
        wp = np.zeros((C, 384), np.float32)
        wp[:, :COLS] = p["lin_p_w"].T[:, 288 * k:288 * (k + 1)]
        d["wp"] = _rbf(wp)
        bp = np.zeros((384,), np.float32)
        bp[:COLS] = p["lin_p_b"][288 * k:288 * (k + 1)]
        d["bp"] = bp
        T = np.zeros((UPR, 4), np.float32)
        for j in range(UPR):
            R = 12 * k - 1 + j
            if R < 0 or R >= 96:
                continue
            s = R * 47.0 / 95.0
            y0 = int(np.floor(s))
            wy = s - y0
            y1 = min(y0 + 1, 47)
            for yi, wv in ((y0, 1.0 - wy), (y1, wy)):
                dd = yi - (6 * k - 2) - (j // 2)
                assert 0 <= dd <= 3, (k, j, yi, dd)
                T[j, dd] += np.float32(wv)
        whx = np.zeros((2, 4, 7, 48), np.float32)
        for q in range(2):
            for dd in range(4):
                for r in range(7):
                    whx[q, dd, r, :] = T[2 * r + q, dd]
        d["whx"] = np.ascontiguousarray(whx.reshape(-1))
        km1, kp1 = max(k - 1, 0), min(k + 1, NCORE - 1)
        meta = np.zeros((1, 8), np.uint32)
        meta[0, 0] = km1 * C * COLS + 4 * 48
        meta[0, 1] = k * C * COLS
        meta[0, 2] = kp1 * C * COLS
        d["meta"] = meta
        in_maps.append(d)
    return in_maps


def kernel(x, params):
    in_maps = host_inputs(x, params)
    nc = _get_program()
    res = run_bass_kernel_spmd(nc, in_maps, core_ids=list(range(NCORE)))
    out = np.empty((C, 96, 96), np.float32)
    for k in range(NCORE):
        out[:, 12 * k:12 * (k + 1), :] = \
            res.results[k]["out"].reshape(C, OUTR, 96)
    return out[None]


# revision 5
# speedup vs baseline: 1.0789x; 1.0789x over previous
"""Trainium2 Bass SPMD kernel for nn_GCM_23390391894818.

8 NeuronCores, one SPMD program; per-core behavior differs only through input
data (slices, per-core interp weights, dynamic pn-window offsets).

Sharding: core k owns 288 tokens (= 6 rows of the 48x48 pooled grid) for all
channel-major linears and the attention (transposed, unnormalized-exp form),
and 12 rows of the 96x96 grid for the upsample+dsc stage.  Collectives:
AllGather Pl/Ph/V (bf16), dyn, pn (fp32); AllReduce for the attention
column-mean.  All matmuls bf16 with fp32 PSUM accumulation; BN folded into
pointwise weights on host; residual/LN/upsample in fp32.
"""

import sys

for _p in ("/opt/trn_rl_repo",):
    if _p not in sys.path:
        sys.path.insert(0, _p)

import numpy as np
import ml_dtypes

import concourse.bass as bass
import concourse.mybir as mybir
import concourse.tile as tile
from concourse import bacc
from concourse.bass_utils import run_bass_kernel_spmd
from concourse.tile_rust import add_dep_helper
from concourse.masks import make_identity

P = 128
C = 2304
CT = C // P            # 18 channel tiles
NCORE = 8
COLS = 288             # own tokens / columns per core (48-res)
WIN48 = 10             # pooled window rows: [6k-2, 6k+8)
WINC = WIN48 * 48      # 480
PNW = 10               # pn window rows48: [6k-2, 6k+8)
UPR = 14               # up window rows96: [12k-1, 12k+13)
OUTR = 12
OUTC = OUTR * 96       # 1152
JC = [(0, 128), (128, 128), (256, 32)]
NCH5 = [(0, 512), (512, 512), (1024, 512), (1536, 512), (2048, 256)]
NCH3 = [(0, 512), (512, 512), (1024, 128)]
SCALE = C ** -0.5
EPS = 1e-5

f32 = mybir.dt.float32
bf16 = mybir.dt.bfloat16
u32 = mybir.dt.uint32
ALU = mybir.AluOpType
ACT = mybir.ActivationFunctionType

BF = np.dtype(ml_dtypes.bfloat16)


def _ap(h, offset, pattern):
    return bass.AP(tensor=h, offset=offset, ap=[list(x) for x in pattern])


def _dyn_ap(h, offset_scalar, pattern):
    return bass.AP(tensor=h, offset=offset_scalar, ap=[list(x) for x in pattern],
                   dep_tracking_offset=0)


def build_program():
    nc = bacc.Bacc(target_bir_lowering=False, trn_type="TRN2")

    # ---------------- external tensors ----------------
    xp_h = nc.dram_tensor("xp", [C, WINC], bf16, kind="ExternalInput")
    xg_h = nc.dram_tensor("xg", [C, COLS], bf16, kind="ExternalInput")
    w_h = {}
    for nm in ("wq", "wk", "wl", "wsh", "wsv", "wch", "wo", "wd"):
        w_h[nm] = nc.dram_tensor(nm, [C, C], bf16, kind="ExternalInput")
    wp_h = nc.dram_tensor("wp", [C, 384], bf16, kind="ExternalInput")
    b_h = {}
    for nm in ("bq", "bk", "bl", "bsh", "bsv", "bch", "bd"):
        b_h[nm] = nc.dram_tensor(nm, [C], f32, kind="ExternalInput")
    bp_h = nc.dram_tensor("bp", [384], f32, kind="ExternalInput")
    obrow_h = nc.dram_tensor("obrow", [P, C], f32, kind="ExternalInput")
    lng_h = nc.dram_tensor("lng", [P, C], f32, kind="ExternalInput")
    lnb_h = nc.dram_tensor("lnb", [P, C], f32, kind="ExternalInput")
    dw5h_h = nc.dram_tensor("dw5h", [C, 5], f32, kind="ExternalInput")
    dw5v_h = nc.dram_tensor("dw5v", [C, 5], f32, kind="ExternalInput")
    dw9_h = nc.dram_tensor("dw9", [C, 9], f32, kind="ExternalInput")
    dwbh_h = nc.dram_tensor("dwbh", [C], f32, kind="ExternalInput")
    dwbv_h = nc.dram_tensor("dwbv", [C], f32, kind="ExternalInput")
    dwbd_h = nc.dram_tensor("dwbd", [C], f32, kind="ExternalInput")
    whx_h = nc.dram_tensor("whx", [2 * 3 * 7 * 48], f32, kind="ExternalInput")
    wwx_h = nc.dram_tensor("wwx", [2 * 3 * 14 * 48], f32, kind="ExternalInput")
    meta_h = nc.dram_tensor("meta", [1, 8], u32, kind="ExternalInput")

    out_h = nc.dram_tensor("out", [C, OUTC], f32, kind="ExternalOutput")

    # ---------------- internal DRAM ----------------
    agpl_i = nc.dram_tensor("agpl_i", [C, COLS], bf16)
    agpl_o = nc.dram_tensor("agpl_o", [NCORE * C, COLS], bf16, addr_space="Shared")
    agph_i = nc.dram_tensor("agph_i", [C, COLS], bf16)
    agph_o = nc.dram_tensor("agph_o", [NCORE * C, COLS], bf16, addr_space="Shared")
    agv_i = nc.dram_tensor("agv_i", [COLS, C], bf16)
    agv_o = nc.dram_tensor("agv_o", [NCORE * COLS, C], bf16, addr_space="Shared")
    ard_i = nc.dram_tensor("ard_i", [C], f32)
    ard_o = nc.dram_tensor("ard_o", [C], f32, addr_space="Shared")
    agdy_i = nc.dram_tensor("agdy_i", [COLS], f32)
    agdy_o = nc.dram_tensor("agdy_o", [NCORE * COLS], f32, addr_space="Shared")
    agpn_i1 = nc.dram_tensor("agpn_i1", [C // 2, COLS], f32)
    agpn_o1 = nc.dram_tensor("agpn_o1", [NCORE * C // 2, COLS], f32, addr_space="Shared")
    agpn_i2 = nc.dram_tensor("agpn_i2", [C // 2, COLS], f32)
    agpn_o2 = nc.dram_tensor("agpn_o2", [NCORE * C // 2, COLS], f32, addr_space="Shared")
    zd_d = nc.dram_tensor("zd", [2, 384], f32)
    vown_d = nc.dram_tensor("vown", [COLS, C], f32)
    upb_d = nc.dram_tensor("upb", [C, OUTC], f32)

    RG = [list(range(NCORE))]

    sp = nc.engines[mybir.EngineType.SP]
    moff = []
    for i in range(3):
        r = sp.alloc_register(f"meta_{i}")
        sp.reg_load(r, meta_h[0:1, i:i + 1])
        moff.append(nc.snap(r, donate=True, min_val=0,
                            max_val=NCORE * C * COLS // 2))

    with tile.TileContext(nc) as tc:
      with tc.tile_pool(name="consts", bufs=1) as cp:
        bt = {}
        for nm in ("bq", "bk", "bl", "bsh", "bsv", "bch", "bd"):
            t = cp.tile([P, CT], f32, tag=f"c_{nm}")
            nc.sync.dma_start(t[:], _ap(b_h[nm], 0, [(1, P), (P, CT)]))
            bt[nm] = t
        bp_t = cp.tile([P, 3], f32, tag="c_bp")
        nc.sync.dma_start(bp_t[:], _ap(bp_h, 0, [(1, P), (P, 3)]))
        dw9_t = cp.tile([P, CT, 9], f32, tag="c_dw9")
        nc.sync.dma_start(dw9_t[:], _ap(dw9_h, 0, [(9, P), (9 * P, CT), (1, 9)]))
        dwbd_t = cp.tile([P, CT], f32, tag="c_dwbd")
        nc.sync.dma_start(dwbd_t[:], _ap(dwbd_h, 0, [(1, P), (P, CT)]))
        ones_t = cp.tile([P, 1], bf16, tag="c_ones")
        nc.vector.memset(ones_t[:], 1.0)
        ident_t = cp.tile([P, P], f32, tag="c_ident")
        make_identity(nc, ident_t[:])

        with tc.tile_pool(name="poolB", bufs=1) as pb:
          plph_t = pb.tile([P, CT, COLS], f32, tag="plph")
          u_t = pb.tile([P, CT, COLS], bf16, tag="u")
          with tc.tile_pool(name="poolB2", bufs=1) as pb2:
            plbf_t = pb2.tile([P, CT, COLS], bf16, tag="plbf")
            phbf_t = pb2.tile([P, CT, COLS], bf16, tag="phbf")

            with tc.tile_pool(name="poolA", bufs=1) as pa:
              xp_t = pa.tile([P, CT, WINC], bf16, tag="xp")
              nc.sync.dma_start(
                  xp_t[:], _ap(xp_h, 0, [(WINC, P), (WINC * P, CT), (1, WINC)]))
              xg_t = pa.tile([P, CT, COLS], bf16, tag="xg")
              nc.sync.dma_start(
                  xg_t[:], _ap(xg_h, 0, [(COLS, P), (COLS * P, CT), (1, COLS)]))

              # ===== phase V: V_token (own rows, token-major) =====
              vw_dmas = []
              with tc.tile_pool(name="poolV", bufs=1) as pv, \
                   tc.tile_pool(name="poolVs", bufs=3) as pvs, \
                   tc.tile_pool(name="psV", bufs=4, space="PSUM") as psv:
                vf_t = pv.tile([P, 3, C], f32, tag="vf")
                vbf_t = pv.tile([P, 3, C], bf16, tag="vbf")
                obrow_t = pv.tile([P, C], f32, tag="obrow")
                nc.sync.dma_start(obrow_t[:], obrow_h.ap())
                for n0, nw in NCH5:
                    pss = [psv.tile([P, 512], f32, tag="ps", name=f"psv_{n0}_{_j}") for _j in range(3)]
                    for k in range(CT):
                        wt = pvs.tile([P, 512], bf16, tag="vw")
                        nc.sync.dma_start(
                            wt[:, :nw],
                            _ap(w_h["wo"], k * P * C + n0, [(C, P), (1, nw)]))
                        for j, (j0, nj) in enumerate(JC):
                            nc.tensor.matmul(
                                pss[j][:nj, :nw],
                                xp_t[:, k, 96 + j0:96 + j0 + nj],
                                wt[:, :nw], start=(k == 0), stop=(k == CT - 1))
                    for j, (j0, nj) in enumerate(JC):
                        nc.vector.tensor_add(vf_t[:nj, j, n0:n0 + nw],
                                             pss[j][:nj, :nw],
                                             obrow_t[:nj, n0:n0 + nw])
                        nc.vector.tensor_copy(vbf_t[:nj, j, n0:n0 + nw],
                                              vf_t[:nj, j, n0:n0 + nw])
                dvs = []
                for j, (j0, nj) in enumerate(JC):
                    dvs.append(nc.sync.dma_start(
                        _ap(agv_i, j0 * C, [(C, nj), (1, C)]), vbf_t[:nj, j, :]))
                    vw_dmas.append(nc.sync.dma_start(
                        _ap(vown_d, j0 * C, [(C, nj), (1, C)]), vf_t[:nj, j, :]))
                c_v = nc.gpsimd.collective_compute(
                    "AllGather", ALU.bypass, replica_groups=RG,
                    ins=[agv_i.ap().opt()], outs=[agv_o.ap().opt()])
                for dv in dvs:
                    add_dep_helper(c_v.ins, dv.ins)

              # ===== phase 1+2: depthwise 5-taps, gelu, six linears =====
              with tc.tile_pool(name="poolL", bufs=1) as pl, \
                   tc.tile_pool(name="poolLs", bufs=3) as pls, \
                   tc.tile_pool(name="psL", bufs=4, space="PSUM") as psl:
                gh_t = pl.tile([P, CT, COLS], bf16, tag="gh")
                gv_t = pl.tile([P, CT, COLS], bf16, tag="gv")
                dw5h_t = pl.tile([P, CT, 5], f32, tag="dw5h")
                nc.sync.dma_start(
                    dw5h_t[:], _ap(dw5h_h, 0, [(5, P), (5 * P, CT), (1, 5)]))
                dw5v_t = pl.tile([P, CT, 5], f32, tag="dw5v")
                nc.sync.dma_start(
                    dw5v_t[:], _ap(dw5v_h, 0, [(5, P), (5 * P, CT), (1, 5)]))
                dwbh_t = pl.tile([P, CT], f32, tag="dwbh")
                nc.sync.dma_start(dwbh_t[:], _ap(dwbh_h, 0, [(1, P), (P, CT)]))
                dwbv_t = pl.tile([P, CT], f32, tag="dwbv")
                nc.sync.dma_start(dwbv_t[:], _ap(dwbv_h, 0, [(1, P), (P, CT)]))

                for t in range(CT):
                    xpw = xp_t[:, t, :].rearrange("p (r w) -> p r w", w=48)
                    acc = pls.tile([P, 6, 48], bf16, tag="dwacc")
                    tmp = pls.tile([P, 6, 48], bf16, tag="dwtmp")
                    for tap in range(5):
                        src = xpw[:, tap:tap + 6, :]
                        if tap == 0:
                            nc.vector.tensor_scalar(
                                acc[:], src, dw5h_t[:, t, 0:1], None, ALU.mult)
                        else:
                            nc.vector.tensor_scalar(
                                tmp[:], src, dw5h_t[:, t, tap:tap + 1], None,
                                ALU.mult)
                            nc.vector.tensor_add(acc[:], acc[:], tmp[:])
                    nc.scalar.activation(
                        gh_t[:, t, :].rearrange("p (r w) -> p r w", w=48), acc[:],
                        ACT.Gelu, bias=dwbh_t[:, t:t + 1])
                    acc2 = pls.tile([P, 6, 48], bf16, tag="dwacc2")
                    for tap in (2, 0, 1, 3, 4):
                        s = tap - 2
                        j0 = max(0, -s)
                        w = 48 - abs(s)
                        src = xpw[:, 2:8, j0 + s:j0 + s + w]
                        if tap == 2:
                            nc.vector.tensor_scalar(
                                acc2[:], src, dw5v_t[:, t, tap:tap + 1], None,
                                ALU.mult)
                            continue
                        tm2 = tmp[:, :, j0:j0 + w]
                        nc.vector.tensor_scalar(
                            tm2, src, dw5v_t[:, t, tap:tap + 1], None, ALU.mult)
                        nc.vector.tensor_add(acc2[:, :, j0:j0 + w],
                                             acc2[:, :, j0:j0 + w], tm2)
                    nc.scalar.activation(
                        gv_t[:, t, :].rearrange("p (r w) -> p r w", w=48), acc2[:],
                        ACT.Gelu, bias=dwbv_t[:, t:t + 1])

                sig_t = pl.tile([P, CT, COLS], bf16, tag="sig")
                xv_t = pl.tile([P, CT, COLS], bf16, tag="xv")
                sum_t = pl.tile([P, CT, COLS], bf16, tag="sumhv")

                def colblock(wh, rhs_t, epilogue):
                    for m in range(CT):
                        wt = pls.tile([P, CT, P], bf16, tag="wstream")
                        nc.sync.dma_start(
                            wt[:], _ap(wh, m * P, [(C, P), (C * P, CT), (1, P)]))
                        ps = psl.tile([P, 512], f32, tag="ps")
                        for k in range(CT):
                            nc.tensor.matmul(ps[:, :COLS], wt[:, k, :],
                                             rhs_t[:, k, :], start=(k == 0),
                                             stop=(k == CT - 1))
                        epilogue(m, ps[:, :COLS])

                def ep_q(m, ps):
                    nc.scalar.activation(sig_t[:, m, :], ps, ACT.Sigmoid,
                                         bias=bt["bq"][:, m:m + 1])
                colblock(w_h["wq"], xg_t, ep_q)

                def ep_k(m, ps):
                    tm = pls.tile([P, COLS], bf16, tag="ep_tmp")
                    nc.scalar.activation(tm[:], ps, ACT.Identity,
                                         bias=bt["bk"][:, m:m + 1])
                    nc.vector.tensor_mul(tm[:], tm[:], sig_t[:, m, :])
                    nc.vector.tensor_add(xv_t[:, m, :], tm[:], xg_t[:, m, :])
                colblock(w_h["wk"], xg_t, ep_k)

                def ep_l(m, ps):
                    nc.scalar.activation(plph_t[:, m, :], ps, ACT.Identity,
                                         bias=bt["bl"][:, m:m + 1])
                    nc.vector.tensor_copy(plbf_t[:, m, :], plph_t[:, m, :])
                colblock(w_h["wl"], xv_t, ep_l)
                d1 = nc.sync.dma_start(
                    _ap(agpl_i, 0, [(COLS, P), (COLS * P, CT), (1, COLS)]),
                    plbf_t[:])
                c_pl = nc.gpsimd.collective_compute(
                    "AllGather", ALU.bypass, replica_groups=RG,
                    ins=[agpl_i.ap().opt()], outs=[agpl_o.ap().opt()])
                add_dep_helper(c_pl.ins, d1.ins)

                def ep_sh(m, ps):
                    nc.scalar.activation(sum_t[:, m, :], ps, ACT.Identity,
                                         bias=bt["bsh"][:, m:m + 1])
                colblock(w_h["wsh"], gh_t, ep_sh)

                def ep_sv(m, ps):
                    tm = pls.tile([P, COLS], bf16, tag="ep_tmp")
                    nc.scalar.activation(tm[:], ps, ACT.Identity,
                                         bias=bt["bsv"][:, m:m + 1])
                    nc.vector.tensor_add(sum_t[:, m, :], sum_t[:, m, :], tm[:])
                colblock(w_h["wsv"], gv_t, ep_sv)

                def ep_ch(m, ps):
                    tm = pls.tile([P, COLS], f32, tag="ep_tmp32")
                    nc.scalar.activation(tm[:], ps, ACT.Identity,
                                         bias=bt["bch"][:, m:m + 1])
                    nc.vector.tensor_copy(phbf_t[:, m, :], tm[:])
                    nc.vector.tensor_add(plph_t[:, m, :], plph_t[:, m, :], tm[:])
                colblock(w_h["wch"], sum_t, ep_ch)
                d2 = nc.sync.dma_start(
                    _ap(agph_i, 0, [(COLS, P), (COLS * P, CT), (1, COLS)]),
                    phbf_t[:])
                c_ph = nc.gpsimd.collective_compute(
                    "AllGather", ALU.bypass, replica_groups=RG,
                    ins=[agph_i.ap().opt()], outs=[agph_o.ap().opt()])
                add_dep_helper(c_ph.ins, d2.ins)
            # poolA closed (xp/xg/linear temps freed)

            # ===== phase 3: attention =====
            with tc.tile_pool(name="poolC", bufs=1) as pc, \
                 tc.tile_pool(name="poolCs", bufs=3) as pcs, \
                 tc.tile_pool(name="psC", bufs=4, space="PSUM") as psc, \
                 tc.tile_pool(name="psCs", bufs=2, space="PSUM") as pscs:
                e1_t = pc.tile([P, CT, COLS], bf16, tag="e1")
                e2_t = pc.tile([P, CT, COLS], bf16, tag="e2")

                def eblock(ago, coll, rhs_t, eout):
                    for m in range(CT):
                        wt = pcs.tile([P, CT, P], bf16, tag="estream")
                        c0 = m * P
                        while c0 < (m + 1) * P:
                            r = c0 // COLS
                            ce = min((m + 1) * P, (r + 1) * COLS)
                            dm = nc.sync.dma_start(
                                wt[:, :, c0 - m * P:ce - m * P],
                                _ap(ago, (r * C) * COLS + (c0 - r * COLS),
                                    [(COLS, P), (P * COLS, CT), (1, ce - c0)]))
                            add_dep_helper(dm.ins, coll.ins)
                            c0 = ce
                        ps = psc.tile([P, 512], f32, tag="ps")
                        for k in range(CT):
                            nc.tensor.matmul(ps[:, :COLS], wt[:, k, :],
                                             rhs_t[:, k, :], start=(k == 0),
                                             stop=(k == CT - 1))
                        nc.scalar.activation(eout[:, m, :], ps[:, :COLS],
                                             ACT.Exp, scale=float(SCALE))

                eblock(agpl_o, c_pl, phbf_t, e1_t)
                eblock(agph_o, c_ph, plbf_t, e2_t)

                zc_t = pc.tile([P, 2, 3], f32, tag="zc")
                for ei, et in ((0, e1_t), (1, e2_t)):
                    for j, (j0, nj) in enumerate(JC):
                        psz = pscs.tile([P, P], f32, tag="small")
                        for k in range(CT):
                            nc.tensor.matmul(psz[:nj, 0:1], et[:, k, j0:j0 + nj],
                                             ones_t[:], start=(k == 0),
                                             stop=(k == CT - 1))
                        nc.vector.tensor_copy(zc_t[:nj, ei, j:j + 1],
                                              psz[:nj, 0:1])
                nc.vector.reciprocal(zc_t[:], zc_t[:])
                dzw = nc.sync.dma_start(
                    _ap(zd_d, 0, [(1, P), (384, 2), (P, 3)]), zc_t[:])
                zr_t = pc.tile([P, 2, COLS], f32, tag="zrow")
                dzr = nc.sync.dma_start(
                    zr_t[:], _ap(zd_d, 0, [(0, P), (384, 2), (1, COLS)]))
                add_dep_helper(dzr.ins, dzw.ins)
                zb_t = pc.tile([P, 2, COLS], bf16, tag="zrowb")
                nc.vector.tensor_copy(zb_t[:], zr_t[:])

                dsum_t = pc.tile([P, CT], f32, tag="dsum")
                for k in range(CT):
                    tmu = pcs.tile([P, COLS], bf16, tag="utmp")
                    nc.vector.tensor_mul(u_t[:, k, :], e1_t[:, k, :],
                                         zb_t[:, 0, :])
                    nc.vector.tensor_mul(tmu[:], e2_t[:, k, :], zb_t[:, 1, :])
                    nc.vector.tensor_add(u_t[:, k, :], u_t[:, k, :], tmu[:])
                    nc.vector.tensor_reduce(dsum_t[:, k:k + 1], u_t[:, k, :],
                                            axis=mybir.AxisListType.X,
                                            op=ALU.add)
                ddw = nc.sync.dma_start(_ap(ard_i, 0, [(1, P), (P, CT)]),
                                        dsum_t[:])
                c_d = nc.gpsimd.collective_compute(
                    "AllReduce", ALU.add, replica_groups=RG,
                    ins=[ard_i.ap().opt()], outs=[ard_o.ap().opt()])
                add_dep_helper(c_d.ins, ddw.ins)

                dT_t = pc.tile([P, CT], f32, tag="dT")
                drd = nc.sync.dma_start(dT_t[:], _ap(ard_o, 0, [(1, P), (P, CT)]))
                add_dep_helper(drd.ins, c_d.ins)
                dbf_t = pc.tile([P, CT], bf16, tag="dbf")
                nc.vector.tensor_scalar(dbf_t[:], dT_t[:], 1.0 / C, None,
                                        ALU.mult)
                wpt = pc.tile([P, CT, 384], bf16, tag="wpt")
                nc.sync.dma_start(
                    wpt[:], _ap(wp_h, 0, [(384, P), (384 * P, CT), (1, 384)]))
                dyv_t = pc.tile([P, 3], f32, tag="dyv")
                for j, (j0, nj) in enumerate(JC):
                    psd = pscs.tile([P, P], f32, tag="small")
                    for k in range(CT):
                        nc.tensor.matmul(psd[:, 0:1], wpt[:, k, j0:j0 + P],
                                         dbf_t[:, k:k + 1], start=(k == 0),
                                         stop=(k == CT - 1))
                    nc.vector.tensor_copy(dyv_t[:, j:j + 1], psd[:, 0:1])
                nc.vector.tensor_add(dyv_t[:], dyv_t[:], bp_t[:])
                dyws = []
                for j, (j0, nj) in enumerate(JC):
                    dyws.append(nc.sync.dma_start(
                        _ap(agdy_i, j0, [(1, nj)]), dyv_t[:nj, j:j + 1]))
                c_dy = nc.gpsimd.collective_compute(
                    "AllGather", ALU.bypass, replica_groups=RG,
                    ins=[agdy_i.ap().opt()], outs=[agdy_o.ap().opt()])
                for dyw in dyws:
                    add_dep_helper(c_dy.ins, dyw.ins)

            # ===== phase 4: numer + prompt + LN + pn =====
            with tc.tile_pool(name="poolD", bufs=1) as pd, \
                 tc.tile_pool(name="poolDs", bufs=3) as pds, \
                 tc.tile_pool(name="psD", bufs=4, space="PSUM") as psd4, \
                 tc.tile_pool(name="psDs", bufs=2, space="PSUM") as psds:
                pf_t = pd.tile([P, 3, C], f32, tag="pf")
                pno_t = pd.tile([P, CT, COLS], f32, tag="pno")
                dynb_t = pd.tile([P, C], f32, tag="dynb")
                drb = nc.sync.dma_start(dynb_t[:],
                                        _ap(agdy_o, 0, [(0, P), (1, C)]))
                add_dep_helper(drb.ins, c_dy.ins)
                lng_t = pd.tile([P, C], f32, tag="lng")
                nc.sync.dma_start(lng_t[:], lng_h.ap())
                lnb_t = pd.tile([P, C], f32, tag="lnb")
                nc.sync.dma_start(lnb_t[:], lnb_h.ap())

                for n0, nw in NCH5:
                    pss = [psd4.tile([P, 512], f32, tag="ps", name=f"psd_{n0}_{_j}") for _j in range(3)]
                    for k in range(CT):
                        vt = pds.tile([P, 512], bf16, tag="vstream")
                        r0 = k * P
                        while r0 < (k + 1) * P:
                            r = r0 // COLS
                            re = min((k + 1) * P, (r + 1) * COLS)
                            dm = nc.sync.dma_start(
                                vt[r0 - k * P:re - k * P, :nw],
                                _ap(agv_o, r0 * C + n0, [(C, re - r0), (1, nw)]))
                            add_dep_helper(dm.ins, c_v.ins)
                            r0 = re
                        for j, (j0, nj) in enumerate(JC):
                            nc.tensor.matmul(pss[j][:nj, :nw],
                                             u_t[:, k, j0:j0 + nj], vt[:, :nw],
                                             start=(k == 0), stop=(k == CT - 1))
                    for j, (j0, nj) in enumerate(JC):
                        vr = pds.tile([P, 512], f32, tag="vread")
                        dv = nc.sync.dma_start(
                            vr[:nj, :nw],
                            _ap(vown_d, j0 * C + n0, [(C, nj), (1, nw)]))
                        for wdma in vw_dmas:
                            add_dep_helper(dv.ins, wdma.ins)
                        nc.vector.tensor_mul(pf_t[:nj, j, n0:n0 + nw],
                                             pss[j][:nj, :nw],
                                             dynb_t[:nj, n0:n0 + nw])
                        nc.vector.tensor_add(pf_t[:nj, j, n0:n0 + nw],
                                             pf_t[:nj, j, n0:n0 + nw],
                                             vr[:nj, :nw])

                sq_t = pd.tile([P, C], f32, tag="sq")
                for j, (j0, nj) in enumerate(JC):
                    s1 = pds.tile([P, 1], f32, tag="s1")
                    nc.vector.tensor_reduce(s1[:nj], pf_t[:nj, j, :],
                                            axis=mybir.AxisListType.X,
                                            op=ALU.add)
                    nc.vector.tensor_scalar(s1[:nj], s1[:nj], -1.0 / C, None,
                                            ALU.mult)
                    nc.vector.tensor_scalar(pf_t[:nj, j, :], pf_t[:nj, j, :],
                                            s1[:nj], None, ALU.add)
                    nc.scalar.activation(sq_t[:nj], pf_t[:nj, j, :], ACT.Square)
                    v1 = pds.tile([P, 1], f32, tag="v1")
                    nc.vector.tensor_reduce(v1[:nj], sq_t[:nj],
                                            axis=mybir.AxisListType.X,
                                            op=ALU.add)
                    nc.vector.tensor_scalar(v1[:nj], v1[:nj], 1.0 / C, EPS,
                                            ALU.mult, ALU.add)
                    nc.scalar.activation(v1[:nj], v1[:nj], ACT.Sqrt)
                    nc.vector.reciprocal(v1[:nj], v1[:nj])
                    nc.vector.tensor_scalar(pf_t[:nj, j, :], pf_t[:nj, j, :],
                                            v1[:nj], None, ALU.mult)
                    nc.vector.tensor_mul(pf_t[:nj, j, :], pf_t[:nj, j, :],
                                         lng_t[:nj, :])
                    nc.vector.tensor_add(pf_t[:nj, j, :], pf_t[:nj, j, :],
                                         lnb_t[:nj, :])

                for j, (j0, nj) in enumerate(JC):
                    for t in range(CT):
                        pst = psds.tile([P, P], f32, tag="small")
                        nc.tensor.matmul(pst[:, :nj],
                                         pf_t[:nj, j, t * P:(t + 1) * P],
                                         ident_t[:nj, :nj], is_transpose=True,
                                         start=True, stop=True)
                        nc.vector.tensor_add(pno_t[:, t, j0:j0 + nj],
                                             pst[:, :nj],
                                             plph_t[:, t, j0:j0 + nj])
                dpn1 = nc.sync.dma_start(
                    _ap(agpn_i1, 0, [(COLS, P), (COLS * P, CT // 2), (1, COLS)]),
                    pno_t[:, :CT // 2, :])
                c_pn1 = nc.gpsimd.collective_compute(
                    "AllGather", ALU.bypass, replica_groups=RG,
                    ins=[agpn_i1.ap().opt()], outs=[agpn_o1.ap().opt()])
                add_dep_helper(c_pn1.ins, dpn1.ins)
                dpn2 = nc.sync.dma_start(
                    _ap(agpn_i2, 0, [(COLS, P), (COLS * P, CT // 2), (1, COLS)]),
                    pno_t[:, CT // 2:, :])
                c_pn2 = nc.gpsimd.collective_compute(
                    "AllGather", ALU.bypass, replica_groups=RG,
                    ins=[agpn_i2.ap().opt()], outs=[agpn_o2.ap().opt()])
                add_dep_helper(c_pn2.ins, dpn2.ins)

        # poolB/B2 closed
        # ===== phase 5: upsample + dsc =====
        with tc.tile_pool(name="poolE", bufs=1) as pe, \
             tc.tile_pool(name="poolEs", bufs=2) as pes, \
             tc.tile_pool(name="poolEw", bufs=3) as pew, \
             tc.tile_pool(name="psE", bufs=4, space="PSUM") as pse:
            g_t = pe.tile([P, CT, OUTC], bf16, tag="g")
            whx_t = pe.tile([P, 2016], f32, tag="whx")
            nc.sync.dma_start(whx_t[:], _ap(whx_h, 0, [(0, P), (1, 2016)]))
            wwx_t = pe.tile([P, 4032], f32, tag="wwx")
            nc.sync.dma_start(wwx_t[:], _ap(wwx_h, 0, [(0, P), (1, 4032)]))
            pnw_a = pe.tile([P, CT // 2, PNW, 48], f32, tag="pnwina")
            pnw_b = pe.tile([P, CT // 2, PNW, 48], f32, tag="pnwinb")
            pieces = [(0, 2), (2, 6), (8, 2)]
            for half, (pnw_h, ago_h, c_h) in enumerate(
                    [(pnw_a, agpn_o1, c_pn1), (pnw_b, agpn_o2, c_pn2)]):
                for (i0, ln), off in zip(pieces, moff):
                    dm = nc.sync.dma_start(
                        pnw_h[:, :, i0:i0 + ln, :],
                        _dyn_ap(ago_h, off,
                                [(COLS, P), (P * COLS, CT // 2), (48, ln),
                                 (1, 48)]))
                    add_dep_helper(dm.ins, c_h.ins)

            whx4 = whx_t[:].rearrange("p (g r w) -> p g r w", g=6, w=48)
            wwx4 = wwx_t[:].rearrange("p (g r w) -> p g r w", g=6, w=48)
            upw_dmas = []
            for t in range(CT):
                t96 = pes.tile([P, UPR, 50], f32, tag="t96")
                nc.vector.memset(t96[:, :, 0:1], 0.0)
                nc.vector.memset(t96[:, :, 49:50], 0.0)
                t96i = t96[:].rearrange("p (r two) c -> p two r c", two=2)
                tmh = pes.tile([P, 7, 48], f32, tag="tmh")
                for q2 in range(2):
                    dst = t96i[:, q2, :, 1:49]
                    for d in range(3):
                        pnw_h = pnw_a if t < CT // 2 else pnw_b
                        src = pnw_h[:, t % (CT // 2), q2 + d:q2 + d + 7, :]
                        wsl = whx4[:, q2 * 3 + d, :, :]
                        if d == 0:
                            nc.vector.tensor_mul(dst, src, wsl)
                        else:
                            nc.vector.tensor_mul(tmh[:], src, wsl)
                            nc.vector.tensor_add(dst, dst, tmh[:])
                up = pes.tile([P, UPR, 96], f32, tag="up")
                upi = up[:].rearrange("p r (c two) -> p two r c", two=2)
                tmw = pes.tile([P, UPR, 48], f32, tag="tmw")
                for q2 in range(2):
                    dst = upi[:, q2, :, :]
                    for i, d in enumerate((0, 1) if q2 == 0 else (1, 2)):
                        src = t96[:, :, d:d + 48]
                        wsl = wwx4[:, q2 * 3 + d, :, :]
                        if i == 0:
                            nc.vector.tensor_mul(dst, src, wsl)
                        else:
                            nc.vector.tensor_mul(tmw[:], src, wsl)
                            nc.vector.tensor_add(dst, dst, tmw[:])
                upw = nc.sync.dma_start(
                    _ap(upb_d, t * P * OUTC, [(OUTC, P), (1, OUTC)]),
                    up[:, 1:13, :])
                upw_dmas.append(upw)
                upb16 = pes.tile([P, UPR, 96], bf16, tag="upb16")
                nc.vector.tensor_copy(upb16[:], up[:])
                acc = pes.tile([P, OUTR, 96], bf16, tag="dacc")
                tmd = pes.tile([P, OUTR, 96], bf16, tag="dtmp")
                for dy, dx in ((0, 0), (-1, -1), (-1, 0), (-1, 1), (0, -1),
                               (0, 1), (1, -1), (1, 0), (1, 1)):
                    tap = (dy + 1) * 3 + (dx + 1)
                    c0 = max(0, -dx)
                    w = 96 - abs(dx)
                    src = upb16[:, 1 + dy:13 + dy, c0 + dx:c0 + dx + w]
                    if (dy, dx) == (0, 0):
                        nc.vector.tensor_scalar(
                            acc[:], src, dw9_t[:, t, tap:tap + 1], None,
                            ALU.mult)
                        continue
                    tms = tmd[:, :, c0:c0 + w]
                    nc.vector.tensor_scalar(
                        tms, src, dw9_t[:, t, tap:tap + 1], None, ALU.mult)
                    nc.vector.tensor_add(acc[:, :, c0:c0 + w],
                                         acc[:, :, c0:c0 + w], tms)
                nc.scalar.activation(
                    g_t[:, t, :].rearrange("p (r w) -> p r w", w=96), acc[:],
                    ACT.Gelu, bias=dwbd_t[:, t:t + 1])

            for m in range(CT):
                wt = pew.tile([P, CT, P], bf16, tag="wstream5")
                nc.sync.dma_start(
                    wt[:], _ap(w_h["wd"], m * P, [(C, P), (C * P, CT), (1, P)]))
                upo = pew.tile([P, OUTC], f32, tag="upo")
                du = nc.sync.dma_start(
                    upo[:], _ap(upb_d, m * P * OUTC, [(OUTC, P), (1, OUTC)]))
                add_dep_helper(du.ins, upw_dmas[m].ins)
                for n0, nw in NCH3:
                    ps = pse.tile([P, 512], f32, tag="ps5")
                    for k in range(CT):
                        nc.tensor.matmul(ps[:, :nw], wt[:, k, :],
                                         g_t[:, k, n0:n0 + nw],
                                         start=(k == 0), stop=(k == CT - 1))
                    of = pew.tile([P, 512], f32, tag="of")
                    nc.scalar.activation(of[:, :nw], ps[:, :nw], ACT.Identity,
                                         bias=bt["bd"][:, m:m + 1])
                    nc.vector.tensor_add(of[:, :nw], of[:, :nw],
                                         upo[:, n0:n0 + nw])
                    nc.sync.dma_start(
                        _ap(out_h, m * P * OUTC + n0, [(OUTC, P), (1, nw)]),
                        of[:, :nw])

    nc.finalize()
    return nc


_prog_cache = {}


def _get_program():
    if "nc" not in _prog_cache:
        _prog_cache["nc"] = build_program()
    return _prog_cache["nc"]


def _rbf(a):
    return np.ascontiguousarray(a).astype(BF)


def host_inputs(x, params):
    p = {k: np.asarray(v, dtype=np.float32) for k, v in params.items()}
    x = np.asarray(x, dtype=np.float32).reshape(C, 96, 96)

    pooled = x.reshape(C, 48, 2, 48, 2).mean(axis=(2, 4))
    m = pooled.mean(axis=(1, 2))

    def bnfold(pfx):
        s = p[pfx + "_bn_g"] / np.sqrt(p[pfx + "_bn_v"] + 1e-5)
        W = p[pfx + "_pw_w"] * s[:, None]
        b = p[pfx + "_pw_b"] * s + p[pfx + "_bn_b"] - p[pfx + "_bn_m"] * s
        return W, b

    Wsh, bsh = bnfold("sch")
    Wsv, bsv = bnfold("scv")
    Wd, bd = bnfold("dsc")

    common = {
        "wq": _rbf(p["q_w"].T), "wk": _rbf(p["k_w"].T),
        "wl": _rbf(p["lin_l_w"].T), "wsh": _rbf(Wsh.T), "wsv": _rbf(Wsv.T),
        "wch": _rbf(p["convh_w"].T), "wo": _rbf(p["lin_o_w"].T),
        "wd": _rbf(Wd.T),
        "bq": p["q_b"], "bk": p["k_b"], "bl": p["lin_l_b"],
        "bsh": bsh, "bsv": bsv, "bch": p["convh_b"], "bd": bd,
        "obrow": np.ascontiguousarray(np.broadcast_to(p["lin_o_b"], (P, C))),
        "lng": np.ascontiguousarray(np.broadcast_to(p["ln_g"], (P, C))),
        "lnb": np.ascontiguousarray(np.broadcast_to(p["ln_b"], (P, C))),
        "dw5h": np.ascontiguousarray(p["sch_dw_w"][:, 0, :, 0]),
        "dw5v": np.ascontiguousarray(p["scv_dw_w"][:, 0, 0, :]),
        "dw9": np.ascontiguousarray(p["dsc_dw_w"][:, 0].reshape(C, 9)),
        "dwbh": p["sch_dw_b"], "dwbv": p["scv_dw_b"], "dwbd": p["dsc_dw_b"],
    }

    wwx = np.zeros((2, 3, 48), np.float32)
    for c in range(96):
        s = c * 47.0 / 95.0
        x0 = int(np.floor(s))
        wx = s - x0
        x1 = min(x0 + 1, 47)
        q, cc = c % 2, c // 2
        for xi, wv in ((x0, 1.0 - wx), (x1, wx)):
            d = xi + 1 - cc
            assert 0 <= d <= 2, (c, xi, cc)
            wwx[q, d, cc] += np.float32(wv)
    common["wwx"] = np.ascontiguousarray(
        np.repeat(wwx[:, :, None, :], UPR, axis=2).reshape(-1))

    in_maps = []
    for k in range(NCORE):
        d = dict(common)
        xpw = np.zeros((C, WIN48, 48), np.float32)
        lo, hi = 6 * k - 2, 6 * k + 8
        vlo, vhi = max(0, lo), min(48, hi)
        xpw[:, vlo - lo:vhi - lo, :] = pooled[:, vlo:vhi, :]
        d["xp"] = _rbf(xpw.reshape(C, WINC))
        d["xg"] = _rbf((m[:, None, None] * pooled[:, 6 * k:6 * k + 6, :])
                       .reshape(C, COLS))
        wp = np.zeros((C, 384), np.float32)
        wp[:, :COLS] = p["lin_p_w"].T[:, 288 * k:288 * (k + 1)]
        d["wp"] = _rbf(wp)
        bp = np.zeros((384,), np.float32)
        bp[:COLS] = p["lin_p_b"][288 * k:288 * (k + 1)]
        d["bp"] = bp
        T = np.zeros((UPR, 3), np.float32)
        for j in range(UPR):
            R = 12 * k - 1 + j
            if R < 0 or R >= 96:
                continue
            s = R * 47.0 / 95.0
            y0 = int(np.floor(s))
            wy = s - y0
            y1 = min(y0 + 1, 47)
            for yi, wv in ((y0, 1.0 - wy), (y1, wy)):
                dd = yi - (6 * k - 2) - (j + 1) // 2
                assert 0 <= dd <= 2, (k, j, yi, dd)
                T[j, dd] += np.float32(wv)
        whx = np.zeros((2, 3, 7, 48), np.float32)
        for q in range(2):
            for dd in range(3):
                for r in range(7):
                    whx[q, dd, r, :] = T[2 * r + q, dd]
        d["whx"] = np.ascontiguousarray(whx.reshape(-1))
        km1, kp1 = max(k - 1, 0), min(k + 1, NCORE - 1)
        meta = np.zeros((1, 8), np.uint32)
        meta[0, 0] = km1 * (C // 2) * COLS + 4 * 48
        meta[0, 1] = k * (C // 2) * COLS
        meta[0, 2] = kp1 * (C // 2) * COLS
        d["meta"] = meta
        in_maps.append(d)
    return in_maps


def kernel(x, params):
    in_maps = host_inputs(x, params)
    nc = _get_program()
    res = run_bass_kernel_spmd(nc, in_maps, core_ids=list(range(NCORE)))
    out = np.empty((C, 96, 96), np.float32)
    for k in range(NCORE):
        out[:, 12 * k:12 * (k + 1), :] = \
            res.results[k]["out"].reshape(C, OUTR, 96)
    return out[None]


# revision 6
# speedup vs baseline: 1.1409x; 1.0574x over previous
"""Trainium2 Bass SPMD kernel for nn_GCM_23390391894818.

8 NeuronCores, one SPMD program; per-core behavior differs only through input
data (slices, per-core interp weights, dynamic pn-window offsets).

Sharding: core k owns 288 tokens (= 6 rows of the 48x48 pooled grid) for all
channel-major linears and the attention (transposed, unnormalized-exp form),
and 12 rows of the 96x96 grid for the upsample+dsc stage.  Collectives:
AllGather Pl/Ph/V (bf16), dyn, pn (fp32); AllReduce for the attention
column-mean.  All matmuls bf16 with fp32 PSUM accumulation; BN folded into
pointwise weights on host; residual/LN/upsample in fp32.
"""

import sys

for _p in ("/opt/trn_rl_repo",):
    if _p not in sys.path:
        sys.path.insert(0, _p)

import numpy as np
import ml_dtypes

import concourse.bass as bass
import concourse.mybir as mybir
import concourse.tile as tile
from concourse import bacc
from concourse.bass_utils import run_bass_kernel_spmd
from concourse.tile_rust import add_dep_helper
from concourse.masks import make_identity

P = 128
C = 2304
CT = C // P            # 18 channel tiles
NCORE = 8
COLS = 288             # own tokens / columns per core (48-res)
WIN48 = 10             # pooled window rows: [6k-2, 6k+8)
WINC = WIN48 * 48      # 480
PNW = 10               # pn window rows48: [6k-2, 6k+8)
UPR = 14               # up window rows96: [12k-1, 12k+13)
OUTR = 12
OUTC = OUTR * 96       # 1152
JC = [(0, 128), (128, 128), (256, 32)]
NCH5 = [(0, 512), (512, 512), (1024, 512), (1536, 512), (2048, 256)]
NCH3 = [(0, 512), (512, 512), (1024, 128)]
SCALE = C ** -0.5
EPS = 1e-5

f32 = mybir.dt.float32
bf16 = mybir.dt.bfloat16
u32 = mybir.dt.uint32
ALU = mybir.AluOpType
ACT = mybir.ActivationFunctionType

BF = np.dtype(ml_dtypes.bfloat16)


def _ap(h, offset, pattern):
    return bass.AP(tensor=h, offset=offset, ap=[list(x) for x in pattern])


def _dyn_ap(h, offset_scalar, pattern):
    return bass.AP(tensor=h, offset=offset_scalar, ap=[list(x) for x in pattern],
                   dep_tracking_offset=0)


def build_program():
    nc = bacc.Bacc(target_bir_lowering=False, trn_type="TRN2")

    # ---------------- external tensors ----------------
    xp_h = nc.dram_tensor("xp", [C, WINC], bf16, kind="ExternalInput")
    xg_h = nc.dram_tensor("xg", [C, COLS], bf16, kind="ExternalInput")
    w_h = {}
    for nm in ("wq", "wk", "wl", "wsh", "wsv", "wch", "wo", "wd"):
        w_h[nm] = nc.dram_tensor(nm, [C, C], bf16, kind="ExternalInput")
    wp_h = nc.dram_tensor("wp", [C, 384], bf16, kind="ExternalInput")
    b_h = {}
    for nm in ("bq", "bk", "bl", "bsh", "bsv", "bch", "bd"):
        b_h[nm] = nc.dram_tensor(nm, [C], f32, kind="ExternalInput")
    bp_h = nc.dram_tensor("bp", [384], f32, kind="ExternalInput")
    obrow_h = nc.dram_tensor("obrow", [P, C], f32, kind="ExternalInput")
    lng_h = nc.dram_tensor("lng", [P, C], f32, kind="ExternalInput")
    lnb_h = nc.dram_tensor("lnb", [P, C], f32, kind="ExternalInput")
    dw5h_h = nc.dram_tensor("dw5h", [C, 5], f32, kind="ExternalInput")
    dw5v_h = nc.dram_tensor("dw5v", [C, 5], f32, kind="ExternalInput")
    dw9_h = nc.dram_tensor("dw9", [C, 9], f32, kind="ExternalInput")
    dwbh_h = nc.dram_tensor("dwbh", [C], f32, kind="ExternalInput")
    dwbv_h = nc.dram_tensor("dwbv", [C], f32, kind="ExternalInput")
    dwbd_h = nc.dram_tensor("dwbd", [C], f32, kind="ExternalInput")
    whx_h = nc.dram_tensor("whx", [2 * 3 * 7 * 48], f32, kind="ExternalInput")
    wwx_h = nc.dram_tensor("wwx", [2 * 3 * 14 * 48], f32, kind="ExternalInput")
    meta_h = nc.dram_tensor("meta", [1, 8], u32, kind="ExternalInput")

    out_h = nc.dram_tensor("out", [C, OUTC], f32, kind="ExternalOutput")

    # ---------------- internal DRAM ----------------
    agpl_i = nc.dram_tensor("agpl_i", [C, COLS], bf16)
    agpl_o = nc.dram_tensor("agpl_o", [NCORE * C, COLS], bf16, addr_space="Shared")
    agph_i = nc.dram_tensor("agph_i", [C, COLS], bf16)
    agph_o = nc.dram_tensor("agph_o", [NCORE * C, COLS], bf16, addr_space="Shared")
    agv_i = nc.dram_tensor("agv_i", [COLS, C], bf16)
    agv_o = nc.dram_tensor("agv_o", [NCORE * COLS, C], bf16, addr_space="Shared")
    ard_i = nc.dram_tensor("ard_i", [C], f32)
    ard_o = nc.dram_tensor("ard_o", [C], f32, addr_space="Shared")
    agdy_i = nc.dram_tensor("agdy_i", [COLS], f32)
    agdy_o = nc.dram_tensor("agdy_o", [NCORE * COLS], f32, addr_space="Shared")
    agpn_i1 = nc.dram_tensor("agpn_i1", [C // 2, COLS], f32)
    agpn_o1 = nc.dram_tensor("agpn_o1", [NCORE * C // 2, COLS], f32, addr_space="Shared")
    agpn_i2 = nc.dram_tensor("agpn_i2", [C // 2, COLS], f32)
    agpn_o2 = nc.dram_tensor("agpn_o2", [NCORE * C // 2, COLS], f32, addr_space="Shared")
    zd_d = nc.dram_tensor("zd", [2, 384], f32)
    vown_d = nc.dram_tensor("vown", [COLS, C], f32)
    upb_d = nc.dram_tensor("upb", [C, OUTC], f32)

    RG = [list(range(NCORE))]

    sp = nc.engines[mybir.EngineType.SP]
    moff = []
    for i in range(3):
        r = sp.alloc_register(f"meta_{i}")
        sp.reg_load(r, meta_h[0:1, i:i + 1])
        moff.append(nc.snap(r, donate=True, min_val=0,
                            max_val=NCORE * C * COLS // 2))

    with tile.TileContext(nc) as tc:
      with tc.tile_pool(name="consts", bufs=1) as cp:
        bt = {}
        for nm in ("bq", "bk", "bl", "bsh", "bsv", "bch", "bd"):
            t = cp.tile([P, CT], f32, tag=f"c_{nm}")
            nc.sync.dma_start(t[:], _ap(b_h[nm], 0, [(1, P), (P, CT)]))
            bt[nm] = t
        bp_t = cp.tile([P, 3], f32, tag="c_bp")
        nc.sync.dma_start(bp_t[:], _ap(bp_h, 0, [(1, P), (P, 3)]))
        dw9_t = cp.tile([P, CT, 9], f32, tag="c_dw9")
        nc.sync.dma_start(dw9_t[:], _ap(dw9_h, 0, [(9, P), (9 * P, CT), (1, 9)]))
        dwbd_t = cp.tile([P, CT], f32, tag="c_dwbd")
        nc.sync.dma_start(dwbd_t[:], _ap(dwbd_h, 0, [(1, P), (P, CT)]))
        ones_t = cp.tile([P, 1], bf16, tag="c_ones")
        nc.vector.memset(ones_t[:], 1.0)
        ident_t = cp.tile([P, P], f32, tag="c_ident")
        make_identity(nc, ident_t[:])

        with tc.tile_pool(name="poolB", bufs=1) as pb:
          plph_t = pb.tile([P, CT, COLS], f32, tag="plph")
          u_t = pb.tile([P, CT, COLS], bf16, tag="u")
          with tc.tile_pool(name="poolB2", bufs=1) as pb2:
            plbf_t = pb2.tile([P, CT, COLS], bf16, tag="plbf")
            phbf_t = pb2.tile([P, CT, COLS], bf16, tag="phbf")

            with tc.tile_pool(name="poolA", bufs=1) as pa:
              xp_t = pa.tile([P, CT, WINC], bf16, tag="xp")
              nc.sync.dma_start(
                  xp_t[:], _ap(xp_h, 0, [(WINC, P), (WINC * P, CT), (1, WINC)]))
              xg_t = pa.tile([P, CT, COLS], bf16, tag="xg")
              nc.sync.dma_start(
                  xg_t[:], _ap(xg_h, 0, [(COLS, P), (COLS * P, CT), (1, COLS)]))

              # ===== phase V: V_token (own rows, token-major) =====
              vw_dmas = []
              with tc.tile_pool(name="poolV", bufs=1) as pv, \
                   tc.tile_pool(name="poolVs", bufs=3) as pvs, \
                   tc.tile_pool(name="psV", bufs=4, space="PSUM") as psv:
                vf_t = pv.tile([P, 3, C], f32, tag="vf")
                vbf_t = pv.tile([P, 3, C], bf16, tag="vbf")
                obrow_t = pv.tile([P, C], f32, tag="obrow")
                nc.sync.dma_start(obrow_t[:], obrow_h.ap())
                for n0, nw in NCH5:
                    pss = [psv.tile([P, 512], f32, tag="ps", name=f"psv_{n0}_{_j}") for _j in range(3)]
                    for k in range(CT):
                        wt = pvs.tile([P, 512], bf16, tag="vw")
                        nc.sync.dma_start(
                            wt[:, :nw],
                            _ap(w_h["wo"], k * P * C + n0, [(C, P), (1, nw)]))
                        for j, (j0, nj) in enumerate(JC):
                            nc.tensor.matmul(
                                pss[j][:nj, :nw],
                                xp_t[:, k, 96 + j0:96 + j0 + nj],
                                wt[:, :nw], start=(k == 0), stop=(k == CT - 1))
                    for j, (j0, nj) in enumerate(JC):
                        nc.vector.tensor_add(vf_t[:nj, j, n0:n0 + nw],
                                             pss[j][:nj, :nw],
                                             obrow_t[:nj, n0:n0 + nw])
                        nc.vector.tensor_copy(vbf_t[:nj, j, n0:n0 + nw],
                                              vf_t[:nj, j, n0:n0 + nw])
                dvs = []
                for j, (j0, nj) in enumerate(JC):
                    dvs.append(nc.sync.dma_start(
                        _ap(agv_i, j0 * C, [(C, nj), (1, C)]), vbf_t[:nj, j, :]))
                    vw_dmas.append(nc.sync.dma_start(
                        _ap(vown_d, j0 * C, [(C, nj), (1, C)]), vf_t[:nj, j, :]))
                c_v = nc.gpsimd.collective_compute(
                    "AllGather", ALU.bypass, replica_groups=RG,
                    ins=[agv_i.ap().opt()], outs=[agv_o.ap().opt()])
                for dv in dvs:
                    add_dep_helper(c_v.ins, dv.ins)

              # ===== phase 1+2: depthwise 5-taps, gelu, six linears =====
              with tc.tile_pool(name="poolL", bufs=1) as pl, \
                   tc.tile_pool(name="poolLs", bufs=3) as pls, \
                   tc.tile_pool(name="psL", bufs=4, space="PSUM") as psl:
                gh_t = pl.tile([P, CT, COLS], bf16, tag="gh")
                gv_t = pl.tile([P, CT, COLS], bf16, tag="gv")
                dw5h_t = pl.tile([P, CT, 5], f32, tag="dw5h")
                nc.sync.dma_start(
                    dw5h_t[:], _ap(dw5h_h, 0, [(5, P), (5 * P, CT), (1, 5)]))
                dw5v_t = pl.tile([P, CT, 5], f32, tag="dw5v")
                nc.sync.dma_start(
                    dw5v_t[:], _ap(dw5v_h, 0, [(5, P), (5 * P, CT), (1, 5)]))
                dwbh_t = pl.tile([P, CT], f32, tag="dwbh")
                nc.sync.dma_start(dwbh_t[:], _ap(dwbh_h, 0, [(1, P), (P, CT)]))
                dwbv_t = pl.tile([P, CT], f32, tag="dwbv")
                nc.sync.dma_start(dwbv_t[:], _ap(dwbv_h, 0, [(1, P), (P, CT)]))

                for t in range(CT):
                    xpw = xp_t[:, t, :].rearrange("p (r w) -> p r w", w=48)
                    acc = pls.tile([P, 6, 48], bf16, tag="dwacc")
                    tmp = pls.tile([P, 6, 48], bf16, tag="dwtmp")
                    for tap in range(5):
                        src = xpw[:, tap:tap + 6, :]
                        if tap == 0:
                            nc.vector.tensor_scalar(
                                acc[:], src, dw5h_t[:, t, 0:1], None, ALU.mult)
                        else:
                            nc.vector.tensor_scalar(
                                tmp[:], src, dw5h_t[:, t, tap:tap + 1], None,
                                ALU.mult)
                            nc.vector.tensor_add(acc[:], acc[:], tmp[:])
                    nc.scalar.activation(
                        gh_t[:, t, :].rearrange("p (r w) -> p r w", w=48), acc[:],
                        ACT.Gelu, bias=dwbh_t[:, t:t + 1])
                    acc2 = pls.tile([P, 6, 48], bf16, tag="dwacc2")
                    for tap in (2, 0, 1, 3, 4):
                        s = tap - 2
                        j0 = max(0, -s)
                        w = 48 - abs(s)
                        src = xpw[:, 2:8, j0 + s:j0 + s + w]
                        if tap == 2:
                            nc.vector.tensor_scalar(
                                acc2[:], src, dw5v_t[:, t, tap:tap + 1], None,
                                ALU.mult)
                            continue
                        tm2 = tmp[:, :, j0:j0 + w]
                        nc.vector.tensor_scalar(
                            tm2, src, dw5v_t[:, t, tap:tap + 1], None, ALU.mult)
                        nc.vector.tensor_add(acc2[:, :, j0:j0 + w],
                                             acc2[:, :, j0:j0 + w], tm2)
                    nc.scalar.activation(
                        gv_t[:, t, :].rearrange("p (r w) -> p r w", w=48), acc2[:],
                        ACT.Gelu, bias=dwbv_t[:, t:t + 1])

                sig_t = pl.tile([P, CT, COLS], bf16, tag="sig")
                xv_t = pl.tile([P, CT, COLS], bf16, tag="xv")
                sum_t = pl.tile([P, CT, COLS], bf16, tag="sumhv")

                def colblock(wh, rhs_t, epilogue):
                    for m in range(CT):
                        wt = pls.tile([P, CT, P], bf16, tag="wstream")
                        nc.sync.dma_start(
                            wt[:], _ap(wh, m * P, [(C, P), (C * P, CT), (1, P)]))
                        ps = psl.tile([P, 512], f32, tag="ps")
                        for k in range(CT):
                            nc.tensor.matmul(ps[:, :COLS], wt[:, k, :],
                                             rhs_t[:, k, :], start=(k == 0),
                                             stop=(k == CT - 1))
                        epilogue(m, ps[:, :COLS])

                def ep_q(m, ps):
                    nc.scalar.activation(sig_t[:, m, :], ps, ACT.Sigmoid,
                                         bias=bt["bq"][:, m:m + 1])
                colblock(w_h["wq"], xg_t, ep_q)

                def ep_k(m, ps):
                    tm = pls.tile([P, COLS], bf16, tag="ep_tmp")
                    nc.scalar.activation(tm[:], ps, ACT.Identity,
                                         bias=bt["bk"][:, m:m + 1])
                    nc.vector.tensor_mul(tm[:], tm[:], sig_t[:, m, :])
                    nc.vector.tensor_add(xv_t[:, m, :], tm[:], xg_t[:, m, :])
                colblock(w_h["wk"], xg_t, ep_k)

                def ep_l(m, ps):
                    nc.scalar.activation(plph_t[:, m, :], ps, ACT.Identity,
                                         bias=bt["bl"][:, m:m + 1])
                    nc.vector.tensor_copy(plbf_t[:, m, :], plph_t[:, m, :])
                colblock(w_h["wl"], xv_t, ep_l)
                d1 = nc.sync.dma_start(
                    _ap(agpl_i, 0, [(COLS, P), (COLS * P, CT), (1, COLS)]),
                    plbf_t[:])
                c_pl = nc.gpsimd.collective_compute(
                    "AllGather", ALU.bypass, replica_groups=RG,
                    ins=[agpl_i.ap().opt()], outs=[agpl_o.ap().opt()])
                add_dep_helper(c_pl.ins, d1.ins)

                def ep_sh(m, ps):
                    nc.scalar.activation(sum_t[:, m, :], ps, ACT.Identity,
                                         bias=bt["bsh"][:, m:m + 1])
                colblock(w_h["wsh"], gh_t, ep_sh)

                def ep_sv(m, ps):
                    tm = pls.tile([P, COLS], bf16, tag="ep_tmp")
                    nc.scalar.activation(tm[:], ps, ACT.Identity,
                                         bias=bt["bsv"][:, m:m + 1])
                    nc.vector.tensor_add(sum_t[:, m, :], sum_t[:, m, :], tm[:])
                colblock(w_h["wsv"], gv_t, ep_sv)

                def ep_ch(m, ps):
                    tm = pls.tile([P, COLS], f32, tag="ep_tmp32")
                    nc.scalar.activation(tm[:], ps, ACT.Identity,
                                         bias=bt["bch"][:, m:m + 1])
                    nc.vector.tensor_copy(phbf_t[:, m, :], tm[:])
                    nc.vector.tensor_add(plph_t[:, m, :], plph_t[:, m, :], tm[:])
                colblock(w_h["wch"], sum_t, ep_ch)
                d2 = nc.sync.dma_start(
                    _ap(agph_i, 0, [(COLS, P), (COLS * P, CT), (1, COLS)]),
                    phbf_t[:])
                c_ph = nc.gpsimd.collective_compute(
                    "AllGather", ALU.bypass, replica_groups=RG,
                    ins=[agph_i.ap().opt()], outs=[agph_o.ap().opt()])
                add_dep_helper(c_ph.ins, d2.ins)
            # poolA closed (xp/xg/linear temps freed)

            # ===== phase 3: attention =====
            with tc.tile_pool(name="poolC", bufs=1) as pc, \
                 tc.tile_pool(name="poolCs", bufs=3) as pcs, \
                 tc.tile_pool(name="psC", bufs=4, space="PSUM") as psc, \
                 tc.tile_pool(name="psCs", bufs=2, space="PSUM") as pscs:
                e1_t = pc.tile([P, CT, COLS], bf16, tag="e1")
                e2_t = pc.tile([P, CT, COLS], bf16, tag="e2")

                def eblock(ago, coll, rhs_t, eout):
                    for m in range(CT):
                        wt = pcs.tile([P, CT, P], bf16, tag="estream")
                        c0 = m * P
                        while c0 < (m + 1) * P:
                            r = c0 // COLS
                            ce = min((m + 1) * P, (r + 1) * COLS)
                            dm = nc.sync.dma_start(
                                wt[:, :, c0 - m * P:ce - m * P],
                                _ap(ago, (r * C) * COLS + (c0 - r * COLS),
                                    [(COLS, P), (P * COLS, CT), (1, ce - c0)]))
                            add_dep_helper(dm.ins, coll.ins)
                            c0 = ce
                        ps = psc.tile([P, 512], f32, tag="ps")
                        for k in range(CT):
                            nc.tensor.matmul(ps[:, :COLS], wt[:, k, :],
                                             rhs_t[:, k, :], start=(k == 0),
                                             stop=(k == CT - 1))
                        nc.scalar.activation(eout[:, m, :], ps[:, :COLS],
                                             ACT.Exp, scale=float(SCALE))

                eblock(agpl_o, c_pl, phbf_t, e1_t)
                eblock(agph_o, c_ph, plbf_t, e2_t)

                zc_t = pc.tile([P, 2, 3], f32, tag="zc")
                for ei, et in ((0, e1_t), (1, e2_t)):
                    for j, (j0, nj) in enumerate(JC):
                        psz = pscs.tile([P, P], f32, tag="small")
                        for k in range(CT):
                            nc.tensor.matmul(psz[:nj, 0:1], et[:, k, j0:j0 + nj],
                                             ones_t[:], start=(k == 0),
                                             stop=(k == CT - 1))
                        nc.vector.tensor_copy(zc_t[:nj, ei, j:j + 1],
                                              psz[:nj, 0:1])
                nc.vector.reciprocal(zc_t[:], zc_t[:])
                dzw = nc.sync.dma_start(
                    _ap(zd_d, 0, [(1, P), (384, 2), (P, 3)]), zc_t[:])
                zr_t = pc.tile([P, 2, COLS], f32, tag="zrow")
                dzr = nc.sync.dma_start(
                    zr_t[:], _ap(zd_d, 0, [(0, P), (384, 2), (1, COLS)]))
                add_dep_helper(dzr.ins, dzw.ins)
                zb_t = pc.tile([P, 2, COLS], bf16, tag="zrowb")
                nc.vector.tensor_copy(zb_t[:], zr_t[:])

                dsum_t = pc.tile([P, CT], f32, tag="dsum")
                for k in range(CT):
                    tmu = pcs.tile([P, COLS], bf16, tag="utmp")
                    nc.vector.tensor_mul(u_t[:, k, :], e1_t[:, k, :],
                                         zb_t[:, 0, :])
                    nc.vector.tensor_mul(tmu[:], e2_t[:, k, :], zb_t[:, 1, :])
                    nc.vector.tensor_add(u_t[:, k, :], u_t[:, k, :], tmu[:])
                    nc.vector.tensor_reduce(dsum_t[:, k:k + 1], u_t[:, k, :],
                                            axis=mybir.AxisListType.X,
                                            op=ALU.add)
                ddw = nc.sync.dma_start(_ap(ard_i, 0, [(1, P), (P, CT)]),
                                        dsum_t[:])
                c_d = nc.gpsimd.collective_compute(
                    "AllReduce", ALU.add, replica_groups=RG,
                    ins=[ard_i.ap().opt()], outs=[ard_o.ap().opt()])
                add_dep_helper(c_d.ins, ddw.ins)

            # ===== phase 4: numer + prompt + LN + pn =====
            with tc.tile_pool(name="poolD", bufs=1) as pd, \
                 tc.tile_pool(name="poolDs", bufs=3) as pds, \
                 tc.tile_pool(name="psD", bufs=4, space="PSUM") as psd4, \
                 tc.tile_pool(name="psDs", bufs=2, space="PSUM") as psds:
                pf_t = pd.tile([P, 3, C], f32, tag="pf")
                pno_t = pd.tile([P, CT, COLS], f32, tag="pno")
                lng_t = pd.tile([P, C], f32, tag="lng")
                nc.sync.dma_start(lng_t[:], lng_h.ap())
                lnb_t = pd.tile([P, C], f32, tag="lnb")
                nc.sync.dma_start(lnb_t[:], lnb_h.ap())

                for n0, nw in NCH5:
                    pss = [psd4.tile([P, 512], f32, tag="ps", name=f"psd_{n0}_{_j}") for _j in range(3)]
                    for k in range(CT):
                        vt = pds.tile([P, 512], bf16, tag="vstream")
                        r0 = k * P
                        while r0 < (k + 1) * P:
                            r = r0 // COLS
                            re = min((k + 1) * P, (r + 1) * COLS)
                            dm = nc.sync.dma_start(
                                vt[r0 - k * P:re - k * P, :nw],
                                _ap(agv_o, r0 * C + n0, [(C, re - r0), (1, nw)]))
                            add_dep_helper(dm.ins, c_v.ins)
                            r0 = re
                        for j, (j0, nj) in enumerate(JC):
                            nc.tensor.matmul(pss[j][:nj, :nw],
                                             u_t[:, k, j0:j0 + nj], vt[:, :nw],
                                             start=(k == 0), stop=(k == CT - 1))
                    for j, (j0, nj) in enumerate(JC):
                        nc.vector.tensor_copy(pf_t[:nj, j, n0:n0 + nw],
                                              pss[j][:nj, :nw])

                # dyn chain (after numer so its collective wait is off the
                # PE critical path)
                dT_t = pd.tile([P, CT], f32, tag="dT")
                drd = nc.sync.dma_start(dT_t[:], _ap(ard_o, 0, [(1, P), (P, CT)]))
                add_dep_helper(drd.ins, c_d.ins)
                dbf_t = pd.tile([P, CT], bf16, tag="dbf")
                nc.vector.tensor_scalar(dbf_t[:], dT_t[:], 1.0 / C, None,
                                        ALU.mult)
                wpt = pd.tile([P, CT, 384], bf16, tag="wpt")
                nc.sync.dma_start(
                    wpt[:], _ap(wp_h, 0, [(384, P), (384 * P, CT), (1, 384)]))
                dyv_t = pd.tile([P, 3], f32, tag="dyv")
                for j, (j0, nj) in enumerate(JC):
                    psd = psds.tile([P, P], f32, tag="small")
                    for k in range(CT):
                        nc.tensor.matmul(psd[:, 0:1], wpt[:, k, j0:j0 + P],
                                         dbf_t[:, k:k + 1], start=(k == 0),
                                         stop=(k == CT - 1))
                    nc.vector.tensor_copy(dyv_t[:, j:j + 1], psd[:, 0:1])
                nc.vector.tensor_add(dyv_t[:], dyv_t[:], bp_t[:])
                dyws = []
                for j, (j0, nj) in enumerate(JC):
                    dyws.append(nc.sync.dma_start(
                        _ap(agdy_i, j0, [(1, nj)]), dyv_t[:nj, j:j + 1]))
                c_dy = nc.gpsimd.collective_compute(
                    "AllGather", ALU.bypass, replica_groups=RG,
                    ins=[agdy_i.ap().opt()], outs=[agdy_o.ap().opt()])
                for dyw in dyws:
                    add_dep_helper(c_dy.ins, dyw.ins)
                dynb_t = pd.tile([P, C], f32, tag="dynb")
                drb = nc.sync.dma_start(dynb_t[:],
                                        _ap(agdy_o, 0, [(0, P), (1, C)]))
                add_dep_helper(drb.ins, c_dy.ins)

                # prompt = dyn * numer + V
                for j, (j0, nj) in enumerate(JC):
                    vr = pds.tile([P, C], f32, tag="vread")
                    dv = nc.sync.dma_start(
                        vr[:nj, :], _ap(vown_d, j0 * C, [(C, nj), (1, C)]))
                    for wdma in vw_dmas:
                        add_dep_helper(dv.ins, wdma.ins)
                    nc.vector.tensor_mul(pf_t[:nj, j, :], pf_t[:nj, j, :],
                                         dynb_t[:nj, :])
                    nc.vector.tensor_add(pf_t[:nj, j, :], pf_t[:nj, j, :],
                                         vr[:nj, :])

                sq_t = pd.tile([P, C], f32, tag="sq")
                for j, (j0, nj) in enumerate(JC):
                    s1 = pds.tile([P, 1], f32, tag="s1")
                    nc.vector.tensor_reduce(s1[:nj], pf_t[:nj, j, :],
                                            axis=mybir.AxisListType.X,
                                            op=ALU.add)
                    nc.vector.tensor_scalar(s1[:nj], s1[:nj], -1.0 / C, None,
                                            ALU.mult)
                    nc.vector.tensor_scalar(pf_t[:nj, j, :], pf_t[:nj, j, :],
                                            s1[:nj], None, ALU.add)
                    nc.scalar.activation(sq_t[:nj], pf_t[:nj, j, :], ACT.Square)
                    v1 = pds.tile([P, 1], f32, tag="v1")
                    nc.vector.tensor_reduce(v1[:nj], sq_t[:nj],
                                            axis=mybir.AxisListType.X,
                                            op=ALU.add)
                    nc.vector.tensor_scalar(v1[:nj], v1[:nj], 1.0 / C, EPS,
                                            ALU.mult, ALU.add)
                    nc.scalar.activation(v1[:nj], v1[:nj], ACT.Sqrt)
                    nc.vector.reciprocal(v1[:nj], v1[:nj])
                    nc.vector.tensor_scalar(pf_t[:nj, j, :], pf_t[:nj, j, :],
                                            v1[:nj], None, ALU.mult)
                    nc.vector.tensor_mul(pf_t[:nj, j, :], pf_t[:nj, j, :],
                                         lng_t[:nj, :])
                    nc.vector.tensor_add(pf_t[:nj, j, :], pf_t[:nj, j, :],
                                         lnb_t[:nj, :])

                for j, (j0, nj) in enumerate(JC):
                    for t in range(CT):
                        pst = psds.tile([P, P], f32, tag="small")
                        nc.tensor.matmul(pst[:, :nj],
                                         pf_t[:nj, j, t * P:(t + 1) * P],
                                         ident_t[:nj, :nj], is_transpose=True,
                                         start=True, stop=True)
                        nc.vector.tensor_add(pno_t[:, t, j0:j0 + nj],
                                             pst[:, :nj],
                                             plph_t[:, t, j0:j0 + nj])
                dpn1 = nc.sync.dma_start(
                    _ap(agpn_i1, 0, [(COLS, P), (COLS * P, CT // 2), (1, COLS)]),
                    pno_t[:, :CT // 2, :])
                c_pn1 = nc.gpsimd.collective_compute(
                    "AllGather", ALU.bypass, replica_groups=RG,
                    ins=[agpn_i1.ap().opt()], outs=[agpn_o1.ap().opt()])
                add_dep_helper(c_pn1.ins, dpn1.ins)
                dpn2 = nc.sync.dma_start(
                    _ap(agpn_i2, 0, [(COLS, P), (COLS * P, CT // 2), (1, COLS)]),
                    pno_t[:, CT // 2:, :])
                c_pn2 = nc.gpsimd.collective_compute(
                    "AllGather", ALU.bypass, replica_groups=RG,
                    ins=[agpn_i2.ap().opt()], outs=[agpn_o2.ap().opt()])
                add_dep_helper(c_pn2.ins, dpn2.ins)

        # poolB/B2 closed
        # ===== phase 5: upsample + dsc =====
        with tc.tile_pool(name="poolE", bufs=1) as pe, \
             tc.tile_pool(name="poolEs", bufs=2) as pes, \
             tc.tile_pool(name="poolEw", bufs=3) as pew, \
             tc.tile_pool(name="psE", bufs=4, space="PSUM") as pse:
            g_t = pe.tile([P, CT, OUTC], bf16, tag="g")
            whx_t = pe.tile([P, 2016], f32, tag="whx")
            nc.sync.dma_start(whx_t[:], _ap(whx_h, 0, [(0, P), (1, 2016)]))
            wwx_t = pe.tile([P, 4032], f32, tag="wwx")
            nc.sync.dma_start(wwx_t[:], _ap(wwx_h, 0, [(0, P), (1, 4032)]))
            pnw_a = pe.tile([P, CT // 2, PNW, 48], f32, tag="pnwina")
            pnw_b = pe.tile([P, CT // 2, PNW, 48], f32, tag="pnwinb")
            pieces = [(0, 2), (2, 6), (8, 2)]
            for half, (pnw_h, ago_h, c_h) in enumerate(
                    [(pnw_a, agpn_o1, c_pn1), (pnw_b, agpn_o2, c_pn2)]):
                for (i0, ln), off in zip(pieces, moff):
                    dm = nc.sync.dma_start(
                        pnw_h[:, :, i0:i0 + ln, :],
                        _dyn_ap(ago_h, off,
                                [(COLS, P), (P * COLS, CT // 2), (48, ln),
                                 (1, 48)]))
                    add_dep_helper(dm.ins, c_h.ins)

            whx4 = whx_t[:].rearrange("p (g r w) -> p g r w", g=6, w=48)
            wwx4 = wwx_t[:].rearrange("p (g r w) -> p g r w", g=6, w=48)
            upw_dmas = []
            for t in range(CT):
                t96 = pes.tile([P, UPR, 50], f32, tag="t96")
                nc.vector.memset(t96[:, :, 0:1], 0.0)
                nc.vector.memset(t96[:, :, 49:50], 0.0)
                t96i = t96[:].rearrange("p (r two) c -> p two r c", two=2)
                tmh = pes.tile([P, 7, 48], f32, tag="tmh")
                for q2 in range(2):
                    dst = t96i[:, q2, :, 1:49]
                    for d in range(3):
                        pnw_h = pnw_a if t < CT // 2 else pnw_b
                        src = pnw_h[:, t % (CT // 2), q2 + d:q2 + d + 7, :]
                        wsl = whx4[:, q2 * 3 + d, :, :]
                        if d == 0:
                            nc.vector.tensor_mul(dst, src, wsl)
                        else:
                            nc.vector.tensor_mul(tmh[:], src, wsl)
                            nc.vector.tensor_add(dst, dst, tmh[:])
                up = pes.tile([P, UPR, 96], f32, tag="up")
                upi = up[:].rearrange("p r (c two) -> p two r c", two=2)
                tmw = pes.tile([P, UPR, 48], f32, tag="tmw")
                ve = nc.gpsimd if (t % 3 == 2) else nc.vector
                for q2 in range(2):
                    dst = upi[:, q2, :, :]
                    for i, d in enumerate((0, 1) if q2 == 0 else (1, 2)):
                        src = t96[:, :, d:d + 48]
                        wsl = wwx4[:, q2 * 3 + d, :, :]
                        if i == 0:
                            ve.tensor_mul(dst, src, wsl)
                        else:
                            ve.tensor_mul(tmw[:], src, wsl)
                            ve.tensor_add(dst, dst, tmw[:])
                upw = nc.sync.dma_start(
                    _ap(upb_d, t * P * OUTC, [(OUTC, P), (1, OUTC)]),
                    up[:, 1:13, :])
                upw_dmas.append(upw)
                upb16 = pes.tile([P, UPR, 96], bf16, tag="upb16")
                nc.vector.tensor_copy(upb16[:], up[:])
                acc = pes.tile([P, OUTR, 96], bf16, tag="dacc")
                tmd = pes.tile([P, OUTR, 96], bf16, tag="dtmp")
                for dy, dx in ((0, 0), (-1, -1), (-1, 0), (-1, 1), (0, -1),
                               (0, 1), (1, -1), (1, 0), (1, 1)):
                    tap = (dy + 1) * 3 + (dx + 1)
                    c0 = max(0, -dx)
                    w = 96 - abs(dx)
                    src = upb16[:, 1 + dy:13 + dy, c0 + dx:c0 + dx + w]
                    if (dy, dx) == (0, 0):
                        nc.scalar.activation(acc[:], src, ACT.Copy,
                                             scale=dw9_t[:, t, tap:tap + 1])
                        continue
                    tms = tmd[:, :, c0:c0 + w]
                    nc.scalar.activation(tms, src, ACT.Copy,
                                         scale=dw9_t[:, t, tap:tap + 1])
                    nc.vector.tensor_add(acc[:, :, c0:c0 + w],
                                         acc[:, :, c0:c0 + w], tms)
                nc.scalar.activation(
                    g_t[:, t, :].rearrange("p (r w) -> p r w", w=96), acc[:],
                    ACT.Gelu, bias=dwbd_t[:, t:t + 1])

            for m in range(CT):
                wt = pew.tile([P, CT, P], bf16, tag="wstream5")
                nc.sync.dma_start(
                    wt[:], _ap(w_h["wd"], m * P, [(C, P), (C * P, CT), (1, P)]))
                upo = pew.tile([P, OUTC], f32, tag="upo")
                du = nc.sync.dma_start(
                    upo[:], _ap(upb_d, m * P * OUTC, [(OUTC, P), (1, OUTC)]))
                add_dep_helper(du.ins, upw_dmas[m].ins)
                for n0, nw in NCH3:
                    ps = pse.tile([P, 512], f32, tag="ps5")
                    for k in range(CT):
                        nc.tensor.matmul(ps[:, :nw], wt[:, k, :],
                                         g_t[:, k, n0:n0 + nw],
                                         start=(k == 0), stop=(k == CT - 1))
                    of = pew.tile([P, 512], f32, tag="of")
                    nc.scalar.activation(of[:, :nw], ps[:, :nw], ACT.Identity,
                                         bias=bt["bd"][:, m:m + 1])
                    nc.vector.tensor_add(of[:, :nw], of[:, :nw],
                                         upo[:, n0:n0 + nw])
                    nc.sync.dma_start(
                        _ap(out_h, m * P * OUTC + n0, [(OUTC, P), (1, nw)]),
                        of[:, :nw])

    nc.finalize()
    return nc


_prog_cache = {}


def _get_program():
    if "nc" not in _prog_cache:
        _prog_cache["nc"] = build_program()
    return _prog_cache["nc"]


def _rbf(a):
    return np.ascontiguousarray(a).astype(BF)


def host_inputs(x, params):
    p = {k: np.asarray(v, dtype=np.float32) for k, v in params.items()}
    x = np.asarray(x, dtype=np.float32).reshape(C, 96, 96)

    pooled = x.reshape(C, 48, 2, 48, 2).mean(axis=(2, 4))
    m = pooled.mean(axis=(1, 2))

    def bnfold(pfx):
        s = p[pfx + "_bn_g"] / np.sqrt(p[pfx + "_bn_v"] + 1e-5)
        W = p[pfx + "_pw_w"] * s[:, None]
        b = p[pfx + "_pw_b"] * s + p[pfx + "_bn_b"] - p[pfx + "_bn_m"] * s
        return W, b

    Wsh, bsh = bnfold("sch")
    Wsv, bsv = bnfold("scv")
    Wd, bd = bnfold("dsc")

    common = {
        "wq": _rbf(p["q_w"].T), "wk": _rbf(p["k_w"].T),
        "wl": _rbf(p["lin_l_w"].T), "wsh": _rbf(Wsh.T), "wsv": _rbf(Wsv.T),
        "wch": _rbf(p["convh_w"].T), "wo": _rbf(p["lin_o_w"].T),
        "wd": _rbf(Wd.T),
        "bq": p["q_b"], "bk": p["k_b"], "bl": p["lin_l_b"],
        "bsh": bsh, "bsv": bsv, "bch": p["convh_b"], "bd": bd,
        "obrow": np.ascontiguousarray(np.broadcast_to(p["lin_o_b"], (P, C))),
        "lng": np.ascontiguousarray(np.broadcast_to(p["ln_g"], (P, C))),
        "lnb": np.ascontiguousarray(np.broadcast_to(p["ln_b"], (P, C))),
        "dw5h": np.ascontiguousarray(p["sch_dw_w"][:, 0, :, 0]),
        "dw5v": np.ascontiguousarray(p["scv_dw_w"][:, 0, 0, :]),
        "dw9": np.ascontiguousarray(p["dsc_dw_w"][:, 0].reshape(C, 9)),
        "dwbh": p["sch_dw_b"], "dwbv": p["scv_dw_b"], "dwbd": p["dsc_dw_b"],
    }

    wwx = np.zeros((2, 3, 48), np.float32)
    for c in range(96):
        s = c * 47.0 / 95.0
        x0 = int(np.floor(s))
        wx = s - x0
        x1 = min(x0 + 1, 47)
        q, cc = c % 2, c // 2
        for xi, wv in ((x0, 1.0 - wx), (x1, wx)):
            d = xi + 1 - cc
            assert 0 <= d <= 2, (c, xi, cc)
            wwx[q, d, cc] += np.float32(wv)
    common["wwx"] = np.ascontiguousarray(
        np.repeat(wwx[:, :, None, :], UPR, axis=2).reshape(-1))

    in_maps = []
    for k in range(NCORE):
        d = dict(common)
        xpw = np.zeros((C, WIN48, 48), np.float32)
        lo, hi = 6 * k - 2, 6 * k + 8
        vlo, vhi = max(0, lo), min(48, hi)
        xpw[:, vlo - lo:vhi - lo, :] = pooled[:, vlo:vhi, :]
        d["xp"] = _rbf(xpw.reshape(C, WINC))
        d["xg"] = _rbf((m[:, None, None] * pooled[:, 6 * k:6 * k + 6, :])
                       .reshape(C, COLS))
        wp = np.zeros((C, 384), np.float32)
        wp[:, :COLS] = p["lin_p_w"].T[:, 288 * k:288 * (k + 1)]
        d["wp"] = _rbf(wp)
        bp = np.zeros((384,), np.float32)
        bp[:COLS] = p["lin_p_b"][288 * k:288 * (k + 1)]
        d["bp"] = bp
        T = np.zeros((UPR, 3), np.float32)
        for j in range(UPR):
            R = 12 * k - 1 + j
            if R < 0 or R >= 96:
                continue
            s = R * 47.0 / 95.0
            y0 = int(np.floor(s))
            wy = s - y0
            y1 = min(y0 + 1, 47)
            for yi, wv in ((y0, 1.0 - wy), (y1, wy)):
                dd = yi - (6 * k - 2) - (j + 1) // 2
                assert 0 <= dd <= 2, (k, j, yi, dd)
                T[j, dd] += np.float32(wv)
        whx = np.zeros((2, 3, 7, 48), np.float32)
        for q in range(2):
            for dd in range(3):
                for r in range(7):
                    whx[q, dd, r, :] = T[2 * r + q, dd]
        d["whx"] = np.ascontiguousarray(whx.reshape(-1))
        km1, kp1 = max(k - 1, 0), min(k + 1, NCORE - 1)
        meta = np.zeros((1, 8), np.uint32)
        meta[0, 0] = km1 * (C // 2) * COLS + 4 * 48
        meta[0, 1] = k * (C // 2) * COLS
        meta[0, 2] = kp1 * (C // 2) * COLS
        d["meta"] = meta
        in_maps.append(d)
    return in_maps


def kernel(x, params):
    in_maps = host_inputs(x, params)
    nc = _get_program()
    res = run_bass_kernel_spmd(nc, in_maps, core_ids=list(range(NCORE)))
    out = np.empty((C, 96, 96), np.float32)
    for k in range(NCORE):
        out[:, 12 * k:12 * (k + 1), :] = \
            res.results[k]["out"].reshape(C, OUTR, 96)
    return out[None]
